# revision 45
# baseline (speedup 1.0000x reference)
"""Trainium2 Bass kernel: 12-head self-attention (B=8, N=1024, D=768).

Sharding: data-parallel over batch - one batch element per NeuronCore,
weights replicated on all 8 cores, no collectives.

Per-core dataflow (matmuls bf16 operands, fp32 PSUM accumulation).
Heads are processed in PAIRS (2p, 2p+1): head 2p lives on SBUF
partitions 0..63, head 2p+1 on 64..127 of the same qk chunk, so the
K=64 score matmuls of the two heads land in different PE row-groups
(row tiling) and execute CONCURRENTLY; the M=64 PV matmuls of the two
heads land in different PE column-groups (col tiling, out partitions
0..63 vs 64..127 of a shared accumulator) and also run concurrently.
Softmax denominators come from M=1 ones-matmuls batched 4 per window
into distinct 32-wide PSUM column strips (4-way concurrent), PSUM-
accumulated over the pair, combined by one M=2 selector matmul + a
2-row reciprocal, broadcast (gpsimd for head A, a DMA partition hop +
gpsimd for head B), and multiplied into attn_sb.

Scheduling: software pipeline paced by the ACT (exp) stream. PV lags
ST/exp by 4 mt steps. At each pair boundary the NEXT pair's first two
ST pairs are emitted ("bridge") before this pair's tail (4 sums
windows + 4 trailing PVs + normalization), so the exp stream never
starves while the tail occupies the PE. qkT and v chains fill the
remaining PE slack; the output projection is split, with chunks
0..PC-1 run as filler once the needed pairs are normalized (partials
stashed bf16 with bias) and the rest forming a short tail with bf16
stores spread over three DMA queues.

PSUM (8 banks): psA = 3 rotating [128,1024] slots (ST_A/ST_B of the
current mt + one filler-chain/sums slot) = 6 banks; psB = the pair's
PV accumulator = 2 banks.

w_qk is HOST-STAGED to [p, t, c, w] layout so each 128-column t-slice
(the weights one head pair needs) is one contiguous-row DMA; the two
prologue slices go on the scalar queue (cheap issue, the ACTIVATE
stream behind them is not delayed), the rest stream on gpsimd behind
wv in pair order.
"""

from contextlib import ExitStack

import numpy as np
import ml_dtypes

import concourse.bacc as bacc
import concourse.bass as bass
import concourse.mybir as mybir
import concourse.tile as tile
from concourse.bass_utils import run_bass_kernel_spmd

B, N, D = 8, 1024, 768
H, HD = 12, 64
NP = H // 2            # 6 head pairs
SCALE = HD ** -0.5
KC = D // 128          # 6 contraction chunks of 128
NT = N // 128          # 8 token tiles of 128
F32 = mybir.dt.float32
BF16 = mybir.dt.bfloat16
NCORES = 8
LAG = 4                # PV trails ST/exp by this many mt steps
# proj phase-1 chunk count per token tile: nt 0-2 run chunks 0-3 as
# pair-4 filler (needs norm(3)), nt 3-7 run chunks 0-4 as pair-5
# filler (needs norm(4)); the rest is the tail.
PC = [4, 4, 4, 5, 5, 5, 5, 5]

_CACHE = {}


def _build_nc():
    nc = bacc.Bacc(None, target_bir_lowering=False)
    xT = nc.dram_tensor("xT", [D, N], BF16, kind="ExternalInput")
    # host-staged: [p, t, c, w] -> W_qk[128c+p, 128t+w], flattened [128, 9216]
    w_qk = nc.dram_tensor("w_qk", [128, 2 * KC * KC * 128], BF16, kind="ExternalInput")
    w_v = nc.dram_tensor("w_v", [D, D], BF16, kind="ExternalInput")
    w_p = nc.dram_tensor("w_p", [D, D], BF16, kind="ExternalInput")
    bias = nc.dram_tensor("bias", [1, D], F32, kind="ExternalInput")
    out = nc.dram_tensor("out", [N, D], BF16, kind="ExternalOutput")

    with ExitStack() as ctx:
        tc = ctx.enter_context(tile.TileContext(nc))
        const = ctx.enter_context(tc.tile_pool(name="const", bufs=1))
        work = ctx.enter_context(tc.tile_pool(name="work", bufs=2))
        psA = ctx.enter_context(tc.tile_pool(name="psA", bufs=3, space="PSUM"))
        psB = ctx.enter_context(tc.tile_pool(name="psB", bufs=1, space="PSUM"))

        xT_sb = const.tile([128, KC, N], BF16)
        wqk_sb = const.tile([128, 2 * KC, KC, 128], BF16)   # [p, t, c, w]
        wv_sb = const.tile([128, KC, D], BF16)
        wp_sb = const.tile([128, KC, D], BF16)
        bias_sb = const.tile([128, D], F32)
        qk_sb = const.tile([128, 2 * KC, N], BF16)   # chunks 0-5: qT, 6-11: kT
        v_sb = const.tile([128, NT, D], BF16)        # per-mt v, head-major
        attn_sb = const.tile([128, KC, N], BF16)     # attn_out^T, normalized
        opart_sb = const.tile([128, NT, D], BF16)    # proj partials + bias
        ones_sb = const.tile([128, 1], BF16)
        selw_sb = const.tile([128, 2], BF16)

        # --- input DMAs ---
        TW = KC * 128  # elements per t-slice per partition row
        for c in range(KC):
            nc.sync.dma_start(out=xT_sb[:, c, :], in_=xT[128 * c:128 * (c + 1), :])
        for t in (0, KC):  # prologue q/k slices for pair 0
            nc.scalar.dma_start(
                out=wqk_sb[:, t, :, :], in_=w_qk[:, TW * t:TW * (t + 1)],
            )
        nc.gpsimd.memset(ones_sb, 1.0)
        # selector weights: col 0 picks rows {0,64} (head A strip partials),
        # col 1 picks rows {32,96} (head B).
        nc.gpsimd.memset(selw_sb, 0.0)
        nc.gpsimd.memset(selw_sb[0:1, 0:1], 1.0)
        nc.gpsimd.memset(selw_sb[64:65, 0:1], 1.0)
        nc.gpsimd.memset(selw_sb[32:33, 1:2], 1.0)
        nc.gpsimd.memset(selw_sb[96:97, 1:2], 1.0)
        # Everything not needed in the first ~25us rides the SYNC ring
        # BEHIND xT: ring order is FIFO, so these can't steal the shared
        # ~330GB/s AXI path from xT + the prologue qk slices during warmup
        # (the Tile scheduler reorders instructions, so emission order
        # alone cannot enforce this).
        for c in range(KC):
            nc.sync.dma_start(out=wv_sb[:, c, :], in_=w_v[128 * c:128 * (c + 1), :])
        for p in range(1, KC):  # remaining qk slices, pair order
            for t in (p, KC + p):
                nc.sync.dma_start(
                    out=wqk_sb[:, t, :, :], in_=w_qk[:, TW * t:TW * (t + 1)],
                )
        for c in range(KC):
            nc.sync.dma_start(out=wp_sb[:, c, :], in_=w_p[128 * c:128 * (c + 1), :])
        bap = bias[:, :]
        bias_bcast = bass.AP(
            tensor=bap.tensor, offset=bap.offset,
            ap=[[0, 128]] + list(bap.ap)[1:],
        )
        nc.sync.dma_start(out=bias_sb, in_=bias_bcast)

        v4 = v_sb.rearrange("p t (h e) -> p t h e", e=HD)

        def qkT_ops(t):
            """Closures: 6 accumulation-chunk matmul pairs + the cast copy,
            for interleaving as PE filler inside a pair's mt loop."""
            ps_qk = psA.tile([128, N], F32, tag="ps", name="ps_qk")
            ops = []
            for c in range(KC):
                def chunk(c=c, ps_qk=ps_qk):
                    for s in range(2):
                        nc.tensor.matmul(
                            ps_qk[:, 512 * s:512 * (s + 1)],
                            lhsT=wqk_sb[:, t, c, :],
                            rhs=xT_sb[:, c, 512 * s:512 * (s + 1)],
                            start=(c == 0), stop=(c == KC - 1),
                        )
                ops.append(chunk)

            def fin(ps_qk=ps_qk):
                nc.vector.tensor_copy(out=qk_sb[:, t, :], in_=ps_qk)
            ops.append(fin)
            return ops

        def v_ops(mt):
            ps_v = psA.tile([128, N], F32, tag="ps", name="ps_v")
            ops = []
            for c in range(KC):
                def chunk(c=c, ps_v=ps_v):
                    for lo, sz in ((0, 512), (512, 256)):
                        nc.tensor.matmul(
                            ps_v[:, lo:lo + sz],
                            lhsT=xT_sb[:, c, 128 * mt:128 * (mt + 1)],
                            rhs=wv_sb[:, c, lo:lo + sz],
                            start=(c == 0), stop=(c == KC - 1),
                        )
                ops.append(chunk)

            def fin(ps_v=ps_v):
                nc.vector.tensor_copy(
                    out=v4[:, mt, :, :],
                    in_=ps_v[:, 0:D].rearrange("p (h e) -> p h e", e=HD),
                )
            ops.append(fin)
            return ops

        def proj1_ops(nt):
            """Proj phase 1: contraction chunks 0..PC[nt]-1 + bias, stashed
            bf16. Only legal once pairs 0..PC[nt]-1 are normalized."""
            pc = PC[nt]
            ps_p = psA.tile([128, N], F32, tag="ps", name="ps_p1")
            ops = []
            for c in range(pc):
                def chunk(c=c, ps_p=ps_p):
                    for lo, sz in ((0, 512), (512, 256)):
                        nc.tensor.matmul(
                            ps_p[:, lo:lo + sz],
                            lhsT=attn_sb[:, c, 128 * nt:128 * (nt + 1)],
                            rhs=wp_sb[:, c, lo:lo + sz],
                            start=(c == 0), stop=(c == pc - 1),
                        )
                ops.append(chunk)

            def fin(ps_p=ps_p):
                nc.vector.tensor_add(
                    out=opart_sb[:, nt, :], in0=ps_p[:, 0:D], in1=bias_sb,
                )
            ops.append(fin)
            return ops

        def emit_proj2(nt, store_q):
            pc = PC[nt]
            ps_p = psA.tile([128, N], F32, tag="ps", name="ps_p2")
            for c in range(pc, KC):
                for lo, sz in ((0, 512), (512, 256)):
                    nc.tensor.matmul(
                        ps_p[:, lo:lo + sz],
                        lhsT=attn_sb[:, c, 128 * nt:128 * (nt + 1)],
                        rhs=wp_sb[:, c, lo:lo + sz],
                        start=(c == pc), stop=(c == KC - 1),
                    )
            o_sb = work.tile([128, D], BF16, tag="o_sb", name="o_sb", bufs=3)
            nc.vector.tensor_add(out=o_sb, in0=ps_p[:, 0:D], in1=opart_sb[:, nt, :])
            store_q.dma_start(out=out[128 * nt:128 * (nt + 1), :], in_=o_sb)

        def emit_ST_pair(p, mt):
            """Concurrent K=64 score matmuls for heads 2p (rows 0:64, PE row
            tile 0) and 2p+1 (rows 64:128, row tile 64), then the two exps.
            """
            tq, tk = p, KC + p
            ps_sA = psA.tile([128, N], F32, tag="ps", name="ps_sA")
            ps_sB = psA.tile([128, N], F32, tag="ps", name="ps_sB")
            for s in range(2):
                nc.tensor.matmul(
                    ps_sA[:, 512 * s:512 * (s + 1)],
                    lhsT=qk_sb[0:64, tk, 128 * mt:128 * (mt + 1)],
                    rhs=qk_sb[0:64, tq, 512 * s:512 * (s + 1)],
                    start=True, stop=True,
                )
                nc.tensor.matmul(
                    ps_sB[:, 512 * s:512 * (s + 1)],
                    lhsT=qk_sb[64:128, tk, 128 * mt:128 * (mt + 1)],
                    rhs=qk_sb[64:128, tq, 512 * s:512 * (s + 1)],
                    start=True, stop=True,
                )
            ptA = work.tile([128, N], BF16, tag="pt", name="ptA", bufs=22)
            ptB = work.tile([128, N], BF16, tag="pt", name="ptB", bufs=22)
            nc.scalar.activation(
                out=ptA, in_=ps_sA,
                func=mybir.ActivationFunctionType.Exp, scale=SCALE,
            )
            nc.scalar.activation(
                out=ptB, in_=ps_sB,
                func=mybir.ActivationFunctionType.Exp, scale=SCALE,
            )
            return ptA, ptB

        def emit_PV_pair(p, mt, ptA, ptB, ps_o):
            """Concurrent M=64 PV matmuls: head A -> out partitions 0:64 (PE
            col tile 0), head B -> 64:128 (col tile 64), shared accumulator."""
            hA, hB = 2 * p, 2 * p + 1
            for s in range(2):
                nc.tensor.matmul(
                    ps_o[0:64, 512 * s:512 * (s + 1)],
                    lhsT=v4[:, mt, hA, :],
                    rhs=ptA[:, 512 * s:512 * (s + 1)],
                    start=(mt == 0), stop=(mt == NT - 1),
                )
                nc.tensor.matmul(
                    ps_o[64:128, 512 * s:512 * (s + 1)],
                    lhsT=v4[:, mt, hB, :],
                    rhs=ptB[:, 512 * s:512 * (s + 1)],
                    start=(mt == 0), stop=(mt == NT - 1),
                )

        def emit_sums_window(ps_m, j, pts):
            """4-way concurrent column-strip sums: ones.T @ P for (A,2j)@0,
            (B,2j)@32, (A,2j+1)@64, (B,2j+1)@96, accumulated over windows."""
            ptA0, ptB0 = pts[2 * j]
            ptA1, ptB1 = pts[2 * j + 1]
            quads = ((0, ptA0), (32, ptB0), (64, ptA1), (96, ptB1))
            for s in range(2):
                for strip, pt in quads:
                    nc.tensor.matmul(
                        ps_m[strip:strip + 1, 512 * s:512 * (s + 1)],
                        lhsT=ones_sb[:, 0:1],
                        rhs=pt[:, 512 * s:512 * (s + 1)],
                        start=(j == 0), stop=(j == 3),
                        tile_position=(0, strip),
                    )

        def emit_pair_tail(p, ps_o, pts, cover=()):
            """Pair tail: 4 sums windows sandwiched with the 4 trailing PVs,
            then normalization (selector matmul on the strip copy, 2-row
            reciprocal, broadcasts, multiplies). `cover` closure-lists are
            independent PE work interleaved to hide the path latency."""
            tq = p
            ci = iter(cover)
            ps_m = psA.tile([128, N], F32, tag="ps", name="ps_m")
            emit_sums_window(ps_m, 0, pts)
            emit_PV_pair(p, NT - 4, pts[NT - 4][0], pts[NT - 4][1], ps_o)
            emit_sums_window(ps_m, 1, pts)
            emit_PV_pair(p, NT - 3, pts[NT - 3][0], pts[NT - 3][1], ps_o)
            emit_sums_window(ps_m, 2, pts)
            emit_PV_pair(p, NT - 2, pts[NT - 2][0], pts[NT - 2][1], ps_o)
            emit_PV_pair(p, NT - 1, pts[NT - 1][0], pts[NT - 1][1], ps_o)
            emit_sums_window(ps_m, 3, pts)
            scp = work.tile([128, N], BF16, tag="scp", name="scp")
            nc.vector.tensor_copy(out=scp, in_=ps_m)
            for op in next(ci, []):
                op()
            for s in range(2):
                nc.tensor.matmul(
                    ps_m[0:2, 512 * s:512 * (s + 1)],
                    lhsT=selw_sb[:, 0:2],
                    rhs=scp[:, 512 * s:512 * (s + 1)],
                    start=True, stop=True,
                )
            for op in next(ci, []):
                op()
            rec2 = work.tile([2, N], F32, tag="rec", name="rec2", bufs=2)
            nc.vector.reciprocal_approx_fast(out=rec2, in_=ps_m[0:2, :])
            rbA = work.tile([128, N], F32, tag="rb", name="rbA")
            rbB = work.tile([128, N], F32, tag="rb", name="rbB")
            nc.gpsimd.partition_broadcast(rbA[0:64, :], rec2[0:1, :])
            # partition_broadcast reads base partition 0 only; hop head B's
            # reciprocal row down from partition 1 via DMA first.
            recB = work.tile([1, N], F32, tag="recB", name="recB", bufs=2)
            nc.sync.dma_start(out=recB, in_=rec2[1:2, :])
            nc.gpsimd.partition_broadcast(rbB, recB)
            for t in ci:
                for op in t:
                    op()
            nc.vector.tensor_mul(
                out=attn_sb[0:64, tq, :], in0=ps_o[0:64, :], in1=rbA[0:64, :],
            )
            nc.vector.tensor_mul(
                out=attn_sb[64:128, tq, :], in0=ps_o[64:128, :], in1=rbB[64:128, :],
            )

        # ---- schedule ----
        # PE warm-up: the HAM p-state ladder needs several us of continuous
        # matmul activity to reach full clock, and the PE otherwise idles
        # from engine-init (~7us) until the first inputs land (~12us).
        # Burn that window on dummy matmuls so the prologue runs warm.
        warm_sb = const.tile([128, 512], BF16)
        nc.gpsimd.memset(warm_sb, 1.0)
        ps_w = psA.tile([128, N], F32, tag="ps", name="ps_warm")
        for _ in range(10):
            nc.tensor.matmul(
                ps_w[:, 0:512], lhsT=warm_sb[:, 0:128], rhs=warm_sb[:, 0:512],
                start=True, stop=True,
            )
        # Prologue: q/k chains for pair 0, chunk-interleaved to match the
        # DMA arrival order of xT chunks.
        ops0, ops6 = qkT_ops(0), qkT_ops(KC)
        for c in range(KC):
            ops0[c]()
            ops6[c]()
        ops0[KC]()
        ops6[KC]()

        # PE filler per pair. All v chains must complete within pair 0
        # (pair-0 PVs consume them at mt pace); qkT chains for pair p+1
        # complete within pair p; proj phase 1 per PC[] above (norm(q) for
        # pair q is emitted in q's tail, before the next pair's filler).
        fillers = {pp: [] for pp in range(NP)}
        for mt in range(NT):
            fillers[0] += v_ops(mt)
        fillers[0] += qkT_ops(1) + qkT_ops(KC + 1)
        for pp in range(1, NP - 1):
            fillers[pp] = qkT_ops(pp + 1) + qkT_ops(KC + pp + 1)
        fillers[4] += proj1_ops(0) + proj1_ops(1) + proj1_ops(2)
        fillers[5] = proj1_ops(3) + proj1_ops(4) + proj1_ops(5)

        bridge = []   # pts of the next pair's first mts, pre-emitted
        for p in range(NP):
            ps_o = psB.tile([128, N], F32, tag="pso", name="ps_o")
            fl = fillers[p]
            fi = 0
            pts = bridge
            lo = 1 if p == 0 else 3
            for mt in range(len(pts), NT):
                ptA, ptB = emit_ST_pair(p, mt)
                pts.append((ptA, ptB))
                if mt >= LAG:
                    k = mt - LAG
                    emit_PV_pair(p, k, pts[k][0], pts[k][1], ps_o)
                if mt >= lo:
                    want = min(len(fl), ((mt - lo + 1) * len(fl) + NT - lo - 1)
                               // (NT - lo))
                    want = max(want, fi)
                    while fi < want:
                        fl[fi]()
                        fi += 1
            while fi < len(fl):
                fl[fi]()
                fi += 1
            # bridge the exp stream: next pair's first two ST pairs run
            # while this pair's tail occupies the PE.
            if p < NP - 1:
                bridge = [emit_ST_pair(p + 1, 0), emit_ST_pair(p + 1, 1),
                          emit_ST_pair(p + 1, 2)]
                emit_pair_tail(p, ps_o, pts)
            else:
                emit_pair_tail(
                    p, ps_o, pts,
                    cover=(proj1_ops(6), proj1_ops(7)),
                )

        store_qs = [nc.sync, nc.scalar, nc.gpsimd]
        for nt in range(NT):
            emit_proj2(nt, store_qs[nt % 3])

    nc.compile()
    return nc


def _get_nc():
    if "nc" not in _CACHE:
        _CACHE["nc"] = _build_nc()
    return _CACHE["nc"]


def _make_in_maps(x, W_qkv, W_proj, b_proj):
    bf = ml_dtypes.bfloat16
    x = np.asarray(x, dtype=np.float32)
    W_qkv = np.asarray(W_qkv, dtype=np.float32)
    W_proj = np.asarray(W_proj, dtype=np.float32)
    b_proj = np.asarray(b_proj, dtype=np.float32)
    # stage w_qk to [p, t, c, w]: each t-slice is one contiguous-row DMA
    w_qk = np.ascontiguousarray(
        W_qkv[:, :2 * D].reshape(KC, 128, 2 * KC, 128)
        .transpose(1, 2, 0, 3).reshape(128, 2 * KC * KC * 128)
    ).astype(bf)
    w_v = np.ascontiguousarray(W_qkv[:, 2 * D:]).astype(bf)
    w_p = W_proj.astype(bf)
    bias = b_proj.reshape(1, D)
    return [
        {
            "xT": np.ascontiguousarray(x[b].T).astype(bf),
            "w_qk": w_qk,
            "w_v": w_v,
            "w_p": w_p,
            "bias": bias,
        }
        for b in range(NCORES)
    ]


def run(x, W_qkv, W_proj, b_proj, trace=False):
    nc = _get_nc()
    in_maps = _make_in_maps(x, W_qkv, W_proj, b_proj)
    res = run_bass_kernel_spmd(nc, in_maps, core_ids=list(range(NCORES)), trace=trace)
    out = np.stack([res.results[b]["out"] for b in range(NCORES)], axis=0)
    return out.astype(np.float32), res


def kernel(x, W_qkv, W_proj, b_proj):
    out, _ = run(x, W_qkv, W_proj, b_proj, trace=False)
    return out


# revision 46
# speedup vs baseline: 1.1675x; 1.1675x over previous
"""Trainium2 Bass kernel: 12-head self-attention (B=8, N=1024, D=768).

Sharding: data-parallel over batch - one batch element per NeuronCore,
weights replicated on all 8 cores, no collectives.

Per-core dataflow (matmuls bf16 operands, fp32 PSUM accumulation).
Heads are processed in PAIRS (2p, 2p+1): head 2p lives on SBUF
partitions 0..63, head 2p+1 on 64..127 of the same qk chunk, so the
K=64 score matmuls of the two heads land in different PE row-groups
(row tiling) and execute CONCURRENTLY; the M=64 PV matmuls of the two
heads land in different PE column-groups (col tiling, out partitions
0..63 vs 64..127 of a shared accumulator) and also run concurrently.
Softmax denominators come from M=1 ones-matmuls batched 4 per window
into distinct 32-wide PSUM column strips (4-way concurrent), PSUM-
accumulated over the pair, combined by one M=2 selector matmul + a
2-row reciprocal, broadcast (gpsimd for head A, a DMA partition hop +
gpsimd for head B), and multiplied into attn_sb.

Scheduling: software pipeline paced by the ACT (exp) stream. PV lags
ST/exp by 4 mt steps. At each pair boundary the NEXT pair's first two
ST pairs are emitted ("bridge") before this pair's tail (4 sums
windows + 4 trailing PVs + normalization), so the exp stream never
starves while the tail occupies the PE. qkT and v chains fill the
remaining PE slack; the output projection is split, with chunks
0..PC-1 run as filler once the needed pairs are normalized (partials
stashed bf16 with bias) and the rest forming a short tail with bf16
stores spread over three DMA queues.

PSUM (8 banks): psA = 3 rotating [128,1024] slots (ST_A/ST_B of the
current mt + one filler-chain/sums slot) = 6 banks; psB = the pair's
PV accumulator = 2 banks.

w_qk is HOST-STAGED to [p, t, c, w] layout so each 128-column t-slice
(the weights one head pair needs) is one contiguous-row DMA; the two
prologue slices go on the scalar queue (cheap issue, the ACTIVATE
stream behind them is not delayed), the rest stream on gpsimd behind
wv in pair order.
"""

from contextlib import ExitStack

import numpy as np
import ml_dtypes

import concourse.bacc as bacc
import concourse.bass as bass
import concourse.mybir as mybir
import concourse.tile as tile
from concourse.bass_utils import run_bass_kernel_spmd

B, N, D = 8, 1024, 768
H, HD = 12, 64
NP = H // 2            # 6 head pairs
SCALE = HD ** -0.5
KC = D // 128          # 6 contraction chunks of 128
NT = N // 128          # 8 token tiles of 128
F32 = mybir.dt.float32
BF16 = mybir.dt.bfloat16
NCORES = 8
LAG = 4                # PV trails ST/exp by this many mt steps
# proj phase-1 chunk count per token tile: nt 0-2 run chunks 0-3 as
# pair-4 filler (needs norm(3)), nt 3-7 run chunks 0-4 as pair-5
# filler (needs norm(4)); the rest is the tail.
PC = [4, 4, 4, 5, 5, 5, 5, 5]

_CACHE = {}


def _build_nc():
    nc = bacc.Bacc(None, target_bir_lowering=False)
    xT = nc.dram_tensor("xT", [D, N], BF16, kind="ExternalInput")
    # host-staged: [p, t, c, w] -> W_qk[128c+p, 128t+w], flattened [128, 9216]
    w_qk = nc.dram_tensor("w_qk", [128, 2 * KC * KC * 128], BF16, kind="ExternalInput")
    w_v = nc.dram_tensor("w_v", [D, D], BF16, kind="ExternalInput")
    w_p = nc.dram_tensor("w_p", [D, D], BF16, kind="ExternalInput")
    bias = nc.dram_tensor("bias", [1, D], F32, kind="ExternalInput")
    out = nc.dram_tensor("out", [N, D], BF16, kind="ExternalOutput")

    with ExitStack() as ctx:
        tc = ctx.enter_context(tile.TileContext(nc))
        const = ctx.enter_context(tc.tile_pool(name="const", bufs=1))
        work = ctx.enter_context(tc.tile_pool(name="work", bufs=2))
        psA = ctx.enter_context(tc.tile_pool(name="psA", bufs=3, space="PSUM"))
        psB = ctx.enter_context(tc.tile_pool(name="psB", bufs=1, space="PSUM"))

        xT_sb = const.tile([128, KC, N], BF16)
        wqk_sb = const.tile([128, 2 * KC, KC, 128], BF16)   # [p, t, c, w]
        wv_sb = const.tile([128, KC, D], BF16)
        wp_sb = const.tile([128, KC, D], BF16)
        bias_sb = const.tile([128, D], F32)
        qk_sb = const.tile([128, 2 * KC, N], BF16)   # chunks 0-5: qT, 6-11: kT
        v_sb = const.tile([128, NT, D], BF16)        # per-mt v, head-major
        attn_sb = const.tile([128, KC, N], BF16)     # attn_out^T, normalized
        opart_sb = const.tile([128, NT, D], BF16)    # proj partials + bias
        ones_sb = const.tile([128, 1], BF16)
        selw_sb = const.tile([128, 2], BF16)

        # --- input DMAs ---
        TW = KC * 128  # elements per t-slice per partition row
        for c in range(KC):
            nc.sync.dma_start(out=xT_sb[:, c, :], in_=xT[128 * c:128 * (c + 1), :])
        for t in (0, KC):  # prologue q/k slices for pair 0
            nc.scalar.dma_start(
                out=wqk_sb[:, t, :, :], in_=w_qk[:, TW * t:TW * (t + 1)],
            )
        nc.gpsimd.memset(ones_sb, 1.0)
        # selector weights: col 0 picks rows {0,64} (head A strip partials),
        # col 1 picks rows {32,96} (head B).
        nc.gpsimd.memset(selw_sb, 0.0)
        nc.gpsimd.memset(selw_sb[0:1, 0:1], 1.0)
        nc.gpsimd.memset(selw_sb[64:65, 0:1], 1.0)
        nc.gpsimd.memset(selw_sb[32:33, 1:2], 1.0)
        nc.gpsimd.memset(selw_sb[96:97, 1:2], 1.0)
        # Everything not needed in the first ~25us rides the SYNC ring
        # BEHIND xT: ring order is FIFO, so these can't steal the shared
        # ~330GB/s AXI path from xT + the prologue qk slices during warmup
        # (the Tile scheduler reorders instructions, so emission order
        # alone cannot enforce this).
        for c in range(KC):
            nc.sync.dma_start(out=wv_sb[:, c, :], in_=w_v[128 * c:128 * (c + 1), :])
        for p in range(1, KC):  # remaining qk slices, pair order
            for t in (p, KC + p):
                nc.sync.dma_start(
                    out=wqk_sb[:, t, :, :], in_=w_qk[:, TW * t:TW * (t + 1)],
                )
        for c in range(KC):
            nc.sync.dma_start(out=wp_sb[:, c, :], in_=w_p[128 * c:128 * (c + 1), :])
        bap = bias[:, :]
        bias_bcast = bass.AP(
            tensor=bap.tensor, offset=bap.offset,
            ap=[[0, 128]] + list(bap.ap)[1:],
        )
        nc.sync.dma_start(out=bias_sb, in_=bias_bcast)

        v4 = v_sb.rearrange("p t (h e) -> p t h e", e=HD)

        def qkT_ops(t):
            """Closures: 6 accumulation-chunk matmul pairs + the cast copy,
            for interleaving as PE filler inside a pair's mt loop."""
            ps_qk = psA.tile([128, N], F32, tag="ps", name="ps_qk")
            ops = []
            for c in range(KC):
                def chunk(c=c, ps_qk=ps_qk):
                    for s in range(2):
                        nc.tensor.matmul(
                            ps_qk[:, 512 * s:512 * (s + 1)],
                            lhsT=wqk_sb[:, t, c, :],
                            rhs=xT_sb[:, c, 512 * s:512 * (s + 1)],
                            start=(c == 0), stop=(c == KC - 1),
                        )
                ops.append(chunk)

            def fin(ps_qk=ps_qk):
                nc.vector.tensor_copy(out=qk_sb[:, t, :], in_=ps_qk)
            ops.append(fin)
            return ops

        def v_ops(mt):
            ps_v = psA.tile([128, N], F32, tag="ps", name="ps_v")
            ops = []
            for c in range(KC):
                def chunk(c=c, ps_v=ps_v):
                    for lo, sz in ((0, 512), (512, 256)):
                        nc.tensor.matmul(
                            ps_v[:, lo:lo + sz],
                            lhsT=xT_sb[:, c, 128 * mt:128 * (mt + 1)],
                            rhs=wv_sb[:, c, lo:lo + sz],
                            start=(c == 0), stop=(c == KC - 1),
                        )
                ops.append(chunk)

            def fin(ps_v=ps_v):
                nc.vector.tensor_copy(
                    out=v4[:, mt, :, :],
                    in_=ps_v[:, 0:D].rearrange("p (h e) -> p h e", e=HD),
                )
            ops.append(fin)
            return ops

        def proj1_ops(nt):
            """Proj phase 1: contraction chunks 0..PC[nt]-1 + bias, stashed
            bf16. Only legal once pairs 0..PC[nt]-1 are normalized."""
            pc = PC[nt]
            ps_p = psA.tile([128, N], F32, tag="ps", name="ps_p1")
            ops = []
            for c in range(pc):
                def chunk(c=c, ps_p=ps_p):
                    for lo, sz in ((0, 512), (512, 256)):
                        nc.tensor.matmul(
                            ps_p[:, lo:lo + sz],
                            lhsT=attn_sb[:, c, 128 * nt:128 * (nt + 1)],
                            rhs=wp_sb[:, c, lo:lo + sz],
                            start=(c == 0), stop=(c == pc - 1),
                        )
                ops.append(chunk)

            def fin(ps_p=ps_p):
                nc.vector.tensor_add(
                    out=opart_sb[:, nt, :], in0=ps_p[:, 0:D], in1=bias_sb,
                )
            ops.append(fin)
            return ops

        def emit_proj2(nt, store_q):
            pc = PC[nt]
            ps_p = psA.tile([128, N], F32, tag="ps", name="ps_p2")
            for c in range(pc, KC):
                for lo, sz in ((0, 512), (512, 256)):
                    nc.tensor.matmul(
                        ps_p[:, lo:lo + sz],
                        lhsT=attn_sb[:, c, 128 * nt:128 * (nt + 1)],
                        rhs=wp_sb[:, c, lo:lo + sz],
                        start=(c == pc), stop=(c == KC - 1),
                    )
            o_sb = work.tile([128, D], BF16, tag="o_sb", name="o_sb", bufs=3)
            nc.vector.tensor_add(out=o_sb, in0=ps_p[:, 0:D], in1=opart_sb[:, nt, :])
            store_q.dma_start(out=out[128 * nt:128 * (nt + 1), :], in_=o_sb)

        def emit_ST_pair(p, mt):
            """Concurrent K=64 score matmuls for heads 2p (rows 0:64, PE row
            tile 0) and 2p+1 (rows 64:128, row tile 64), then the two exps.
            """
            tq, tk = p, KC + p
            ps_sA = psA.tile([128, N], F32, tag="ps", name="ps_sA")
            ps_sB = psA.tile([128, N], F32, tag="ps", name="ps_sB")
            for s in range(2):
                nc.tensor.matmul(
                    ps_sA[:, 512 * s:512 * (s + 1)],
                    lhsT=qk_sb[0:64, tk, 128 * mt:128 * (mt + 1)],
                    rhs=qk_sb[0:64, tq, 512 * s:512 * (s + 1)],
                    start=True, stop=True,
                )
                nc.tensor.matmul(
                    ps_sB[:, 512 * s:512 * (s + 1)],
                    lhsT=qk_sb[64:128, tk, 128 * mt:128 * (mt + 1)],
                    rhs=qk_sb[64:128, tq, 512 * s:512 * (s + 1)],
                    start=True, stop=True,
                )
            ptA = work.tile([128, N], BF16, tag="pt", name="ptA", bufs=24)
            ptB = work.tile([128, N], BF16, tag="pt", name="ptB", bufs=24)
            nc.scalar.activation(
                out=ptA, in_=ps_sA,
                func=mybir.ActivationFunctionType.Exp, scale=SCALE,
            )
            nc.scalar.activation(
                out=ptB, in_=ps_sB,
                func=mybir.ActivationFunctionType.Exp, scale=SCALE,
            )
            return ptA, ptB

        def emit_PV_pair(p, mt, ptA, ptB, ps_o):
            """Concurrent M=64 PV matmuls: head A -> out partitions 0:64 (PE
            col tile 0), head B -> 64:128 (col tile 64), shared accumulator."""
            hA, hB = 2 * p, 2 * p + 1
            for s in range(2):
                nc.tensor.matmul(
                    ps_o[0:64, 512 * s:512 * (s + 1)],
                    lhsT=v4[:, mt, hA, :],
                    rhs=ptA[:, 512 * s:512 * (s + 1)],
                    start=(mt == 0), stop=(mt == NT - 1),
                )
                nc.tensor.matmul(
                    ps_o[64:128, 512 * s:512 * (s + 1)],
                    lhsT=v4[:, mt, hB, :],
                    rhs=ptB[:, 512 * s:512 * (s + 1)],
                    start=(mt == 0), stop=(mt == NT - 1),
                )

        def emit_sums_window(ps_m, j, pts):
            """4-way concurrent column-strip sums: ones.T @ P for (A,2j)@0,
            (B,2j)@32, (A,2j+1)@64, (B,2j+1)@96, accumulated over windows."""
            ptA0, ptB0 = pts[2 * j]
            ptA1, ptB1 = pts[2 * j + 1]
            quads = ((0, ptA0), (32, ptB0), (64, ptA1), (96, ptB1))
            for s in range(2):
                for strip, pt in quads:
                    nc.tensor.matmul(
                        ps_m[strip:strip + 1, 512 * s:512 * (s + 1)],
                        lhsT=ones_sb[:, 0:1],
                        rhs=pt[:, 512 * s:512 * (s + 1)],
                        start=(j == 0), stop=(j == 3),
                        tile_position=(0, strip),
                    )

        def emit_pair_tail(p, ps_o, pts, cover=()):
            """Pair tail: 4 sums windows sandwiched with the 4 trailing PVs,
            then normalization (selector matmul on the strip copy, 2-row
            reciprocal, broadcasts, multiplies). `cover` closure-lists are
            independent PE work interleaved to hide the path latency."""
            tq = p
            ci = iter(cover)
            ps_m = psA.tile([128, N], F32, tag="ps", name="ps_m")
            emit_sums_window(ps_m, 0, pts)
            emit_PV_pair(p, NT - 4, pts[NT - 4][0], pts[NT - 4][1], ps_o)
            emit_sums_window(ps_m, 1, pts)
            emit_PV_pair(p, NT - 3, pts[NT - 3][0], pts[NT - 3][1], ps_o)
            emit_sums_window(ps_m, 2, pts)
            emit_PV_pair(p, NT - 2, pts[NT - 2][0], pts[NT - 2][1], ps_o)
            emit_PV_pair(p, NT - 1, pts[NT - 1][0], pts[NT - 1][1], ps_o)
            emit_sums_window(ps_m, 3, pts)
            scp = work.tile([128, N], BF16, tag="scp", name="scp")
            nc.vector.tensor_copy(out=scp, in_=ps_m)
            for op in next(ci, []):
                op()
            for s in range(2):
                nc.tensor.matmul(
                    ps_m[0:2, 512 * s:512 * (s + 1)],
                    lhsT=selw_sb[:, 0:2],
                    rhs=scp[:, 512 * s:512 * (s + 1)],
                    start=True, stop=True,
                )
            for op in next(ci, []):
                op()
            rec2 = work.tile([2, N], F32, tag="rec", name="rec2", bufs=2)
            nc.vector.reciprocal_approx_fast(out=rec2, in_=ps_m[0:2, :])
            rbA = work.tile([128, N], F32, tag="rb", name="rbA")
            rbB = work.tile([128, N], F32, tag="rb", name="rbB")
            nc.gpsimd.partition_broadcast(rbA[0:64, :], rec2[0:1, :])
            # partition_broadcast reads base partition 0 only; hop head B's
            # reciprocal row down from partition 1 via DMA first.
            recB = work.tile([1, N], F32, tag="recB", name="recB", bufs=2)
            nc.sync.dma_start(out=recB, in_=rec2[1:2, :])
            nc.gpsimd.partition_broadcast(rbB, recB)
            for t in ci:
                for op in t:
                    op()
            nc.vector.tensor_mul(
                out=attn_sb[0:64, tq, :], in0=ps_o[0:64, :], in1=rbA[0:64, :],
            )
            nc.vector.tensor_mul(
                out=attn_sb[64:128, tq, :], in0=ps_o[64:128, :], in1=rbB[64:128, :],
            )

        # ---- schedule ----
        # PE warm-up: the HAM p-state ladder needs several us of continuous
        # matmul activity to reach full clock, and the PE otherwise idles
        # from engine-init (~7us) until the first inputs land (~12us).
        # Burn that window on dummy matmuls so the prologue runs warm.
        warm_sb = const.tile([128, 512], BF16)
        nc.gpsimd.memset(warm_sb, 1.0)
        ps_w = psA.tile([128, N], F32, tag="ps", name="ps_warm")
        for _ in range(10):
            nc.tensor.matmul(
                ps_w[:, 0:512], lhsT=warm_sb[:, 0:128], rhs=warm_sb[:, 0:512],
                start=True, stop=True,
            )
        # Prologue: q/k chains for pair 0, chunk-interleaved to match the
        # DMA arrival order of xT chunks.
        ops0, ops6 = qkT_ops(0), qkT_ops(KC)
        for c in range(KC):
            ops0[c]()
            ops6[c]()
        ops0[KC]()
        ops6[KC]()

        # PE filler per pair. All v chains must complete within pair 0
        # (pair-0 PVs consume them at mt pace); qkT chains for pair p+1
        # complete within pair p; proj phase 1 per PC[] above (norm(q) for
        # pair q is emitted in q's tail, before the next pair's filler).
        fillers = {pp: [] for pp in range(NP)}
        for mt in range(NT):
            fillers[0] += v_ops(mt)
        fillers[0] += qkT_ops(1) + qkT_ops(KC + 1)
        for pp in range(1, NP - 1):
            fillers[pp] = qkT_ops(pp + 1) + qkT_ops(KC + pp + 1)
        fillers[4] += proj1_ops(0) + proj1_ops(1) + proj1_ops(2)
        fillers[5] = proj1_ops(3) + proj1_ops(4) + proj1_ops(5)

        bridge = []   # pts of the next pair's first mts, pre-emitted
        for p in range(NP):
            ps_o = psB.tile([128, N], F32, tag="pso", name="ps_o")
            fl = fillers[p]
            fi = 0
            pts = bridge
            lo = 1 if p == 0 else 4
            for mt in range(len(pts), NT):
                ptA, ptB = emit_ST_pair(p, mt)
                pts.append((ptA, ptB))
                if mt >= LAG:
                    k = mt - LAG
                    emit_PV_pair(p, k, pts[k][0], pts[k][1], ps_o)
                if mt >= lo:
                    want = min(len(fl), ((mt - lo + 1) * len(fl) + NT - lo - 1)
                               // (NT - lo))
                    want = max(want, fi)
                    while fi < want:
                        fl[fi]()
                        fi += 1
            while fi < len(fl):
                fl[fi]()
                fi += 1
            # bridge the exp stream: next pair's first two ST pairs run
            # while this pair's tail occupies the PE.
            if p < NP - 1:
                bridge = [emit_ST_pair(p + 1, 0), emit_ST_pair(p + 1, 1),
                          emit_ST_pair(p + 1, 2), emit_ST_pair(p + 1, 3)]
                emit_pair_tail(p, ps_o, pts)
            else:
                emit_pair_tail(
                    p, ps_o, pts,
                    cover=(proj1_ops(6), proj1_ops(7)),
                )

        store_qs = [nc.sync, nc.scalar, nc.gpsimd]
        for nt in range(NT):
            emit_proj2(nt, store_qs[nt % 3])

    nc.compile()
    return nc


def _get_nc():
    if "nc" not in _CACHE:
        _CACHE["nc"] = _build_nc()
    return _CACHE["nc"]


def _make_in_maps(x, W_qkv, W_proj, b_proj):
    bf = ml_dtypes.bfloat16
    x = np.asarray(x, dtype=np.float32)
    W_qkv = np.asarray(W_qkv, dtype=np.float32)
    W_proj = np.asarray(W_proj, dtype=np.float32)
    b_proj = np.asarray(b_proj, dtype=np.float32)
    # stage w_qk to [p, t, c, w]: each t-slice is one contiguous-row DMA
    w_qk = np.ascontiguousarray(
        W_qkv[:, :2 * D].reshape(KC, 128, 2 * KC, 128)
        .transpose(1, 2, 0, 3).reshape(128, 2 * KC * KC * 128)
    ).astype(bf)
    w_v = np.ascontiguousarray(W_qkv[:, 2 * D:]).astype(bf)
    w_p = W_proj.astype(bf)
    bias = b_proj.reshape(1, D)
    return [
        {
            "xT": np.ascontiguousarray(x[b].T).astype(bf),
            "w_qk": w_qk,
            "w_v": w_v,
            "w_p": w_p,
            "bias": bias,
        }
        for b in range(NCORES)
    ]


def run(x, W_qkv, W_proj, b_proj, trace=False):
    nc = _get_nc()
    in_maps = _make_in_maps(x, W_qkv, W_proj, b_proj)
    res = run_bass_kernel_spmd(nc, in_maps, core_ids=list(range(NCORES)), trace=trace)
    out = np.stack([res.results[b]["out"] for b in range(NCORES)], axis=0)
    return out.astype(np.float32), res


def kernel(x, W_qkv, W_proj, b_proj):
    out, _ = run(x, W_qkv, W_proj, b_proj, trace=False)
    return out


# revision 49
# speedup vs baseline: 1.1839x; 1.0140x over previous
"""Trainium2 Bass kernel: 12-head self-attention (B=8, N=1024, D=768).

Sharding: data-parallel over batch - one batch element per NeuronCore,
weights replicated on all 8 cores, no collectives.

Per-core dataflow (matmuls bf16 operands, fp32 PSUM accumulation).
Heads are processed in PAIRS (2p, 2p+1): head 2p lives on SBUF
partitions 0..63, head 2p+1 on 64..127 of the same qk chunk, so the
K=64 score matmuls of the two heads land in different PE row-groups
(row tiling) and execute CONCURRENTLY; the M=64 PV matmuls of the two
heads land in different PE column-groups (col tiling, out partitions
0..63 vs 64..127 of a shared accumulator) and also run concurrently.
Softmax denominators come from M=1 ones-matmuls batched 4 per window
into distinct 32-wide PSUM column strips (4-way concurrent), PSUM-
accumulated over the pair, combined by one M=2 selector matmul + a
2-row reciprocal, broadcast (gpsimd for head A, a DMA partition hop +
gpsimd for head B), and multiplied into attn_sb.

Scheduling: software pipeline paced by the ACT (exp) stream. PV lags
ST/exp by 4 mt steps. At each pair boundary the NEXT pair's first two
ST pairs are emitted ("bridge") before this pair's tail (4 sums
windows + 4 trailing PVs + normalization), so the exp stream never
starves while the tail occupies the PE. qkT and v chains fill the
remaining PE slack; the output projection is split, with chunks
0..PC-1 run as filler once the needed pairs are normalized (partials
stashed bf16 with bias) and the rest forming a short tail with bf16
stores spread over three DMA queues.

PSUM (8 banks): psA = 3 rotating [128,1024] slots (ST_A/ST_B of the
current mt + one filler-chain/sums slot) = 6 banks; psB = the pair's
PV accumulator = 2 banks.

w_qk is HOST-STAGED to [p, t, c, w] layout so each 128-column t-slice
(the weights one head pair needs) is one contiguous-row DMA; the two
prologue slices go on the scalar queue (cheap issue, the ACTIVATE
stream behind them is not delayed), the rest stream on gpsimd behind
wv in pair order.
"""

from contextlib import ExitStack

import numpy as np
import ml_dtypes

import concourse.bacc as bacc
import concourse.bass as bass
import concourse.mybir as mybir
import concourse.tile as tile
from concourse.bass_utils import run_bass_kernel_spmd

B, N, D = 8, 1024, 768
H, HD = 12, 64
NP = H // 2            # 6 head pairs
SCALE = HD ** -0.5
KC = D // 128          # 6 contraction chunks of 128
NT = N // 128          # 8 token tiles of 128
F32 = mybir.dt.float32
BF16 = mybir.dt.bfloat16
NCORES = 8
LAG = 4                # PV trails ST/exp by this many mt steps
# proj phase-1 chunk count per token tile: nt 0-2 run chunks 0-3 as
# pair-4 filler (needs norm(3)), nt 3-7 run chunks 0-4 as pair-5
# filler (needs norm(4)); the rest is the tail.
PC = [4, 4, 4, 5, 5, 5, 5, 5]

_CACHE = {}


def _build_nc():
    nc = bacc.Bacc(None, target_bir_lowering=False)
    xT = nc.dram_tensor("xT", [D, N], BF16, kind="ExternalInput")
    # host-staged: [p, t, c, w] -> W_qk[128c+p, 128t+w], flattened [128, 9216]
    w_qk = nc.dram_tensor("w_qk", [128, 2 * KC * KC * 128], BF16, kind="ExternalInput")
    w_v = nc.dram_tensor("w_v", [D, D], BF16, kind="ExternalInput")
    w_p = nc.dram_tensor("w_p", [D, D], BF16, kind="ExternalInput")
    bias = nc.dram_tensor("bias", [1, D], F32, kind="ExternalInput")
    out = nc.dram_tensor("out", [N, D], BF16, kind="ExternalOutput")

    with ExitStack() as ctx:
        tc = ctx.enter_context(tile.TileContext(nc))
        const = ctx.enter_context(tc.tile_pool(name="const", bufs=1))
        work = ctx.enter_context(tc.tile_pool(name="work", bufs=2))
        psA = ctx.enter_context(tc.tile_pool(name="psA", bufs=3, space="PSUM"))
        psB = ctx.enter_context(tc.tile_pool(name="psB", bufs=1, space="PSUM"))

        xT_sb = const.tile([128, KC, N], BF16)
        wqk_sb = const.tile([128, 2 * KC, KC, 128], BF16)   # [p, t, c, w]
        wv_sb = const.tile([128, KC, D], BF16)
        wp_sb = const.tile([128, KC, D], BF16)
        bias_sb = const.tile([128, D], F32)
        qk_sb = const.tile([128, 2 * KC, N], BF16)   # chunks 0-5: qT, 6-11: kT
        v_sb = const.tile([128, NT, D], BF16)        # per-mt v, head-major
        attn_sb = const.tile([128, KC, N], BF16)     # attn_out^T, normalized
        opart_sb = const.tile([128, NT, D], BF16)    # proj partials + bias
        ones_sb = const.tile([128, 1], BF16)
        selw_sb = const.tile([128, 2], BF16)

        # --- input DMAs ---
        TW = KC * 128  # elements per t-slice per partition row
        for c in range(KC):
            nc.sync.dma_start(out=xT_sb[:, c, :], in_=xT[128 * c:128 * (c + 1), :])
        for t in (0, KC):  # prologue q/k slices for pair 0
            nc.scalar.dma_start(
                out=wqk_sb[:, t, :, :], in_=w_qk[:, TW * t:TW * (t + 1)],
            )
        nc.gpsimd.memset(ones_sb, 1.0)
        # selector weights: col 0 picks rows {0,64} (head A strip partials),
        # col 1 picks rows {32,96} (head B).
        nc.gpsimd.memset(selw_sb, 0.0)
        nc.gpsimd.memset(selw_sb[0:1, 0:1], 1.0)
        nc.gpsimd.memset(selw_sb[64:65, 0:1], 1.0)
        nc.gpsimd.memset(selw_sb[32:33, 1:2], 1.0)
        nc.gpsimd.memset(selw_sb[96:97, 1:2], 1.0)
        # Everything not needed in the first ~25us rides the SYNC ring
        # BEHIND xT: ring order is FIFO, so these can't steal the shared
        # ~330GB/s AXI path from xT + the prologue qk slices during warmup
        # (the Tile scheduler reorders instructions, so emission order
        # alone cannot enforce this).
        for c in range(KC):
            nc.sync.dma_start(out=wv_sb[:, c, :], in_=w_v[128 * c:128 * (c + 1), :])
        for p in range(1, KC):  # remaining qk slices, pair order
            for t in (p, KC + p):
                nc.sync.dma_start(
                    out=wqk_sb[:, t, :, :], in_=w_qk[:, TW * t:TW * (t + 1)],
                )
        for c in range(KC):
            nc.sync.dma_start(out=wp_sb[:, c, :], in_=w_p[128 * c:128 * (c + 1), :])
        bap = bias[:, :]
        bias_bcast = bass.AP(
            tensor=bap.tensor, offset=bap.offset,
            ap=[[0, 128]] + list(bap.ap)[1:],
        )
        nc.sync.dma_start(out=bias_sb, in_=bias_bcast)

        v4 = v_sb.rearrange("p t (h e) -> p t h e", e=HD)

        def qkT_ops(t):
            """Closures: 6 accumulation-chunk matmul pairs + the cast copy,
            for interleaving as PE filler inside a pair's mt loop."""
            ps_qk = psA.tile([128, N], F32, tag="ps", name="ps_qk")
            ops = []
            for c in range(KC):
                def chunk(c=c, ps_qk=ps_qk):
                    for s in range(2):
                        nc.tensor.matmul(
                            ps_qk[:, 512 * s:512 * (s + 1)],
                            lhsT=wqk_sb[:, t, c, :],
                            rhs=xT_sb[:, c, 512 * s:512 * (s + 1)],
                            start=(c == 0), stop=(c == KC - 1),
                        )
                ops.append(chunk)

            def fin(ps_qk=ps_qk):
                nc.vector.tensor_copy(out=qk_sb[:, t, :], in_=ps_qk)
            ops.append(fin)
            return ops

        def v_ops(mt):
            ps_v = psA.tile([128, N], F32, tag="ps", name="ps_v")
            ops = []
            for c in range(KC):
                def chunk(c=c, ps_v=ps_v):
                    for lo, sz in ((0, 512), (512, 256)):
                        nc.tensor.matmul(
                            ps_v[:, lo:lo + sz],
                            lhsT=xT_sb[:, c, 128 * mt:128 * (mt + 1)],
                            rhs=wv_sb[:, c, lo:lo + sz],
                            start=(c == 0), stop=(c == KC - 1),
                        )
                ops.append(chunk)

            def fin(ps_v=ps_v):
                nc.vector.tensor_copy(
                    out=v4[:, mt, :, :],
                    in_=ps_v[:, 0:D].rearrange("p (h e) -> p h e", e=HD),
                )
            ops.append(fin)
            return ops

        def proj1_ops(nt):
            """Proj phase 1: contraction chunks 0..PC[nt]-1 + bias, stashed
            bf16. Only legal once pairs 0..PC[nt]-1 are normalized."""
            pc = PC[nt]
            ps_p = psA.tile([128, N], F32, tag="ps", name="ps_p1")
            ops = []
            for c in range(pc):
                def chunk(c=c, ps_p=ps_p):
                    for lo, sz in ((0, 512), (512, 256)):
                        nc.tensor.matmul(
                            ps_p[:, lo:lo + sz],
                            lhsT=attn_sb[:, c, 128 * nt:128 * (nt + 1)],
                            rhs=wp_sb[:, c, lo:lo + sz],
                            start=(c == 0), stop=(c == pc - 1),
                        )
                ops.append(chunk)

            def fin(ps_p=ps_p):
                nc.vector.tensor_add(
                    out=opart_sb[:, nt, :], in0=ps_p[:, 0:D], in1=bias_sb,
                )
            ops.append(fin)
            return ops

        def emit_proj2(nt, store_q):
            pc = PC[nt]
            ps_p = psA.tile([128, N], F32, tag="ps", name="ps_p2")
            for c in range(pc, KC):
                for lo, sz in ((0, 512), (512, 256)):
                    nc.tensor.matmul(
                        ps_p[:, lo:lo + sz],
                        lhsT=attn_sb[:, c, 128 * nt:128 * (nt + 1)],
                        rhs=wp_sb[:, c, lo:lo + sz],
                        start=(c == pc), stop=(c == KC - 1),
                    )
            o_sb = work.tile([128, D], BF16, tag="o_sb", name="o_sb", bufs=3)
            nc.vector.tensor_add(out=o_sb, in0=ps_p[:, 0:D], in1=opart_sb[:, nt, :])
            store_q.dma_start(out=out[128 * nt:128 * (nt + 1), :], in_=o_sb)

        def emit_ST_pair(p, mt):
            """Concurrent K=64 score matmuls for heads 2p (rows 0:64, PE row
            tile 0) and 2p+1 (rows 64:128, row tile 64), then the two exps.
            """
            tq, tk = p, KC + p
            ps_sA = psA.tile([128, N], F32, tag="ps", name="ps_sA")
            ps_sB = psA.tile([128, N], F32, tag="ps", name="ps_sB")
            for s in range(2):
                nc.tensor.matmul(
                    ps_sA[:, 512 * s:512 * (s + 1)],
                    lhsT=qk_sb[0:64, tk, 128 * mt:128 * (mt + 1)],
                    rhs=qk_sb[0:64, tq, 512 * s:512 * (s + 1)],
                    start=True, stop=True,
                )
                nc.tensor.matmul(
                    ps_sB[:, 512 * s:512 * (s + 1)],
                    lhsT=qk_sb[64:128, tk, 128 * mt:128 * (mt + 1)],
                    rhs=qk_sb[64:128, tq, 512 * s:512 * (s + 1)],
                    start=True, stop=True,
                )
            ptA = work.tile([128, N], BF16, tag="pt", name="ptA", bufs=22)
            ptB = work.tile([128, N], BF16, tag="pt", name="ptB", bufs=22)
            nc.scalar.activation(
                out=ptA, in_=ps_sA,
                func=mybir.ActivationFunctionType.Exp, scale=SCALE,
            )
            nc.scalar.activation(
                out=ptB, in_=ps_sB,
                func=mybir.ActivationFunctionType.Exp, scale=SCALE,
            )
            return ptA, ptB

        def emit_PV_pair(p, mt, ptA, ptB, ps_o):
            """Concurrent M=64 PV matmuls: head A -> out partitions 0:64 (PE
            col tile 0), head B -> 64:128 (col tile 64), shared accumulator."""
            hA, hB = 2 * p, 2 * p + 1
            for s in range(2):
                nc.tensor.matmul(
                    ps_o[0:64, 512 * s:512 * (s + 1)],
                    lhsT=v4[:, mt, hA, :],
                    rhs=ptA[:, 512 * s:512 * (s + 1)],
                    start=(mt == 0), stop=(mt == NT - 1),
                )
                nc.tensor.matmul(
                    ps_o[64:128, 512 * s:512 * (s + 1)],
                    lhsT=v4[:, mt, hB, :],
                    rhs=ptB[:, 512 * s:512 * (s + 1)],
                    start=(mt == 0), stop=(mt == NT - 1),
                )

        def emit_sums_window(ps_m, j, pts):
            """4-way concurrent column-strip sums: ones.T @ P for (A,2j)@0,
            (B,2j)@32, (A,2j+1)@64, (B,2j+1)@96, accumulated over windows."""
            ptA0, ptB0 = pts[2 * j]
            ptA1, ptB1 = pts[2 * j + 1]
            quads = ((0, ptA0), (32, ptB0), (64, ptA1), (96, ptB1))
            for s in range(2):
                for strip, pt in quads:
                    nc.tensor.matmul(
                        ps_m[strip:strip + 1, 512 * s:512 * (s + 1)],
                        lhsT=ones_sb[:, 0:1],
                        rhs=pt[:, 512 * s:512 * (s + 1)],
                        start=(j == 0), stop=(j == 3),
                        tile_position=(0, strip),
                    )

        def emit_pair_tail(p, ps_o, pts, cover=()):
            """Pair tail: 4 sums windows sandwiched with the 4 trailing PVs,
            then normalization (selector matmul on the strip copy, 2-row
            reciprocal, broadcasts, multiplies). `cover` closure-lists are
            independent PE work interleaved to hide the path latency."""
            tq = p
            ci = iter(cover)
            ps_m = psA.tile([128, N], F32, tag="ps", name="ps_m")
            emit_sums_window(ps_m, 0, pts)
            emit_PV_pair(p, NT - 4, pts[NT - 4][0], pts[NT - 4][1], ps_o)
            emit_sums_window(ps_m, 1, pts)
            emit_PV_pair(p, NT - 3, pts[NT - 3][0], pts[NT - 3][1], ps_o)
            emit_sums_window(ps_m, 2, pts)
            emit_PV_pair(p, NT - 2, pts[NT - 2][0], pts[NT - 2][1], ps_o)
            emit_PV_pair(p, NT - 1, pts[NT - 1][0], pts[NT - 1][1], ps_o)
            emit_sums_window(ps_m, 3, pts)
            scp = work.tile([128, N], BF16, tag="scp", name="scp")
            nc.vector.tensor_copy(out=scp, in_=ps_m)
            for op in next(ci, []):
                op()
            for s in range(2):
                nc.tensor.matmul(
                    ps_m[0:2, 512 * s:512 * (s + 1)],
                    lhsT=selw_sb[:, 0:2],
                    rhs=scp[:, 512 * s:512 * (s + 1)],
                    start=True, stop=True,
                )
            for op in next(ci, []):
                op()
            rec2 = work.tile([2, N], F32, tag="rec", name="rec2", bufs=2)
            nc.vector.reciprocal_approx_fast(out=rec2, in_=ps_m[0:2, :])
            rbA = work.tile([128, N], F32, tag="rb", name="rbA")
            rbB = work.tile([128, N], F32, tag="rb", name="rbB")
            nc.gpsimd.partition_broadcast(rbA[0:64, :], rec2[0:1, :])
            # partition_broadcast reads base partition 0 only; hop head B's
            # reciprocal row down from partition 1 via DMA first.
            recB = work.tile([1, N], F32, tag="recB", name="recB", bufs=2)
            nc.sync.dma_start(out=recB, in_=rec2[1:2, :])
            nc.gpsimd.partition_broadcast(rbB, recB)
            nc.vector.tensor_mul(
                out=attn_sb[0:64, tq, :], in0=ps_o[0:64, :], in1=rbA[0:64, :],
            )
            nc.vector.tensor_mul(
                out=attn_sb[64:128, tq, :], in0=ps_o[64:128, :], in1=rbB[64:128, :],
            )
            for t in ci:
                for op in t:
                    op()

        # ---- schedule ----
        # PE warm-up: the HAM p-state ladder needs several us of continuous
        # matmul activity to reach full clock, and the PE otherwise idles
        # from engine-init (~7us) until the first inputs land (~12us).
        # Burn that window on dummy matmuls so the prologue runs warm.
        warm_sb = const.tile([128, 512], BF16)
        nc.gpsimd.memset(warm_sb, 1.0)
        ps_w = psA.tile([128, N], F32, tag="ps", name="ps_warm")
        for _ in range(10):
            nc.tensor.matmul(
                ps_w[:, 0:512], lhsT=warm_sb[:, 0:128], rhs=warm_sb[:, 0:512],
                start=True, stop=True,
            )
        # Prologue: q/k chains for pair 0, chunk-interleaved to match the
        # DMA arrival order of xT chunks.
        ops0, ops6 = qkT_ops(0), qkT_ops(KC)
        for c in range(KC):
            ops0[c]()
            ops6[c]()
        ops0[KC]()
        ops6[KC]()

        # PE filler per pair. All v chains must complete within pair 0
        # (pair-0 PVs consume them at mt pace); qkT chains for pair p+1
        # complete within pair p; proj phase 1 per PC[] above (norm(q) for
        # pair q is emitted in q's tail, before the next pair's filler).
        fillers = {pp: [] for pp in range(NP)}
        for mt in range(NT):
            fillers[0] += v_ops(mt)
        fillers[0] += qkT_ops(1) + qkT_ops(KC + 1)
        for pp in range(1, NP - 1):
            fillers[pp] = qkT_ops(pp + 1) + qkT_ops(KC + pp + 1)
        fillers[4] += proj1_ops(0) + proj1_ops(1) + proj1_ops(2)
        fillers[5] = proj1_ops(3) + proj1_ops(4) + proj1_ops(5)

        store_qs = [nc.sync, nc.scalar, nc.gpsimd]
        bridge = []   # pts of the next pair's first mts, pre-emitted
        for p in range(NP):
            ps_o = psB.tile([128, N], F32, tag="pso", name="ps_o")
            fl = fillers[p]
            fi = 0
            pts = bridge
            lo = 1 if p == 0 else 3
            for mt in range(len(pts), NT):
                ptA, ptB = emit_ST_pair(p, mt)
                pts.append((ptA, ptB))
                if mt >= LAG:
                    k = mt - LAG
                    emit_PV_pair(p, k, pts[k][0], pts[k][1], ps_o)
                if mt >= lo:
                    want = min(len(fl), ((mt - lo + 1) * len(fl) + NT - lo - 1)
                               // (NT - lo))
                    want = max(want, fi)
                    while fi < want:
                        fl[fi]()
                        fi += 1
            while fi < len(fl):
                fl[fi]()
                fi += 1
            # bridge the exp stream: next pair's first two ST pairs run
            # while this pair's tail occupies the PE.
            if p < NP - 1:
                bridge = [emit_ST_pair(p + 1, 0), emit_ST_pair(p + 1, 1),
                          emit_ST_pair(p + 1, 2)]
                emit_pair_tail(p, ps_o, pts)
            else:
                emit_pair_tail(
                    p, ps_o, pts,
                    cover=(
                        proj1_ops(6), proj1_ops(7),
                        [lambda nt=nt: emit_proj2(nt, store_qs[nt % 3])
                         for nt in range(3)],
                    ),
                )

        for nt in range(3, NT):
            emit_proj2(nt, store_qs[nt % 3])

    nc.compile()
    return nc


def _get_nc():
    if "nc" not in _CACHE:
        _CACHE["nc"] = _build_nc()
    return _CACHE["nc"]


def _make_in_maps(x, W_qkv, W_proj, b_proj):
    bf = ml_dtypes.bfloat16
    x = np.asarray(x, dtype=np.float32)
    W_qkv = np.asarray(W_qkv, dtype=np.float32)
    W_proj = np.asarray(W_proj, dtype=np.float32)
    b_proj = np.asarray(b_proj, dtype=np.float32)
    # stage w_qk to [p, t, c, w]: each t-slice is one contiguous-row DMA
    w_qk = np.ascontiguousarray(
        W_qkv[:, :2 * D].reshape(KC, 128, 2 * KC, 128)
        .transpose(1, 2, 0, 3).reshape(128, 2 * KC * KC * 128)
    ).astype(bf)
    w_v = np.ascontiguousarray(W_qkv[:, 2 * D:]).astype(bf)
    w_p = W_proj.astype(bf)
    bias = b_proj.reshape(1, D)
    return [
        {
            "xT": np.ascontiguousarray(x[b].T).astype(bf),
            "w_qk": w_qk,
            "w_v": w_v,
            "w_p": w_p,
            "bias": bias,
        }
        for b in range(NCORES)
    ]


def run(x, W_qkv, W_proj, b_proj, trace=False):
    nc = _get_nc()
    in_maps = _make_in_maps(x, W_qkv, W_proj, b_proj)
    res = run_bass_kernel_spmd(nc, in_maps, core_ids=list(range(NCORES)), trace=trace)
    out = np.stack([res.results[b]["out"] for b in range(NCORES)], axis=0)
    return out.astype(np.float32), res


def kernel(x, W_qkv, W_proj, b_proj):
    out, _ = run(x, W_qkv, W_proj, b_proj, trace=False)
    return out


# revision 50
# speedup vs baseline: 1.1869x; 1.0026x over previous
"""Trainium2 Bass kernel: 12-head self-attention (B=8, N=1024, D=768).

Sharding: data-parallel over batch - one batch element per NeuronCore,
weights replicated on all 8 cores, no collectives.

Per-core dataflow (matmuls bf16 operands, fp32 PSUM accumulation).
Heads are processed in PAIRS (2p, 2p+1): head 2p lives on SBUF
partitions 0..63, head 2p+1 on 64..127 of the same qk chunk, so the
K=64 score matmuls of the two heads land in different PE row-groups
(row tiling) and execute CONCURRENTLY; the M=64 PV matmuls of the two
heads land in different PE column-groups (col tiling, out partitions
0..63 vs 64..127 of a shared accumulator) and also run concurrently.
Softmax denominators come from M=1 ones-matmuls batched 4 per window
into distinct 32-wide PSUM column strips (4-way concurrent), PSUM-
accumulated over the pair, combined by one M=2 selector matmul + a
2-row reciprocal, broadcast (gpsimd for head A, a DMA partition hop +
gpsimd for head B), and multiplied into attn_sb.

Scheduling: software pipeline paced by the ACT (exp) stream. PV lags
ST/exp by 4 mt steps. At each pair boundary the NEXT pair's first two
ST pairs are emitted ("bridge") before this pair's tail (4 sums
windows + 4 trailing PVs + normalization), so the exp stream never
starves while the tail occupies the PE. qkT and v chains fill the
remaining PE slack; the output projection is split, with chunks
0..PC-1 run as filler once the needed pairs are normalized (partials
stashed bf16 with bias) and the rest forming a short tail with bf16
stores spread over three DMA queues.

PSUM (8 banks): psA = 3 rotating [128,1024] slots (ST_A/ST_B of the
current mt + one filler-chain/sums slot) = 6 banks; psB = the pair's
PV accumulator = 2 banks.

w_qk is HOST-STAGED to [p, t, c, w] layout so each 128-column t-slice
(the weights one head pair needs) is one contiguous-row DMA; the two
prologue slices go on the scalar queue (cheap issue, the ACTIVATE
stream behind them is not delayed), the rest stream on gpsimd behind
wv in pair order.
"""

from contextlib import ExitStack

import numpy as np
import ml_dtypes

import concourse.bacc as bacc
import concourse.bass as bass
import concourse.mybir as mybir
import concourse.tile as tile
from concourse.bass_utils import run_bass_kernel_spmd

B, N, D = 8, 1024, 768
H, HD = 12, 64
NP = H // 2            # 6 head pairs
SCALE = HD ** -0.5
KC = D // 128          # 6 contraction chunks of 128
NT = N // 128          # 8 token tiles of 128
F32 = mybir.dt.float32
BF16 = mybir.dt.bfloat16
NCORES = 8
LAG = 4                # PV trails ST/exp by this many mt steps
# proj phase-1 chunk count per token tile: nt 0-2 run chunks 0-3 as
# pair-4 filler (needs norm(3)), nt 3-7 run chunks 0-4 as pair-5
# filler (needs norm(4)); the rest is the tail.
PC = [4, 4, 4, 5, 5, 5, 5, 5]

_CACHE = {}


def _build_nc():
    nc = bacc.Bacc(None, target_bir_lowering=False)
    xT = nc.dram_tensor("xT", [D, N], BF16, kind="ExternalInput")
    # host-staged: [p, t, c, w] -> W_qk[128c+p, 128t+w], flattened [128, 9216]
    w_qk = nc.dram_tensor("w_qk", [128, 2 * KC * KC * 128], BF16, kind="ExternalInput")
    w_v = nc.dram_tensor("w_v", [D, D], BF16, kind="ExternalInput")
    w_p = nc.dram_tensor("w_p", [D, D], BF16, kind="ExternalInput")
    bias = nc.dram_tensor("bias", [1, D], F32, kind="ExternalInput")
    out = nc.dram_tensor("out", [N, D], BF16, kind="ExternalOutput")

    with ExitStack() as ctx:
        tc = ctx.enter_context(tile.TileContext(nc))
        const = ctx.enter_context(tc.tile_pool(name="const", bufs=1))
        work = ctx.enter_context(tc.tile_pool(name="work", bufs=2))
        psA = ctx.enter_context(tc.tile_pool(name="psA", bufs=3, space="PSUM"))
        psB = ctx.enter_context(tc.tile_pool(name="psB", bufs=1, space="PSUM"))

        xT_sb = const.tile([128, KC, N], BF16)
        wqk_sb = const.tile([128, 2 * KC, KC, 128], BF16)   # [p, t, c, w]
        wv_sb = const.tile([128, KC, D], BF16)
        wp_sb = const.tile([128, KC, D], BF16)
        bias_sb = const.tile([128, D], F32)
        qk_sb = const.tile([128, 2 * KC, N], BF16)   # chunks 0-5: qT, 6-11: kT
        v_sb = const.tile([128, NT, D], BF16)        # per-mt v, head-major
        attn_sb = const.tile([128, KC, N], BF16)     # attn_out^T, normalized
        opart_sb = const.tile([128, NT, D], BF16)    # proj partials + bias
        ones_sb = const.tile([128, 1], BF16)
        selw_sb = const.tile([128, 2], BF16)

        # --- input DMAs ---
        TW = KC * 128  # elements per t-slice per partition row
        for c in range(KC):
            nc.sync.dma_start(out=xT_sb[:, c, :], in_=xT[128 * c:128 * (c + 1), :])
        for t in (0, KC):  # prologue q/k slices for pair 0
            nc.scalar.dma_start(
                out=wqk_sb[:, t, :, :], in_=w_qk[:, TW * t:TW * (t + 1)],
            )
        nc.gpsimd.memset(ones_sb, 1.0)
        # selector weights: col 0 picks rows {0,64} (head A strip partials),
        # col 1 picks rows {32,96} (head B).
        nc.gpsimd.memset(selw_sb, 0.0)
        nc.gpsimd.memset(selw_sb[0:1, 0:1], 1.0)
        nc.gpsimd.memset(selw_sb[64:65, 0:1], 1.0)
        nc.gpsimd.memset(selw_sb[32:33, 1:2], 1.0)
        nc.gpsimd.memset(selw_sb[96:97, 1:2], 1.0)
        # Everything not needed in the first ~25us rides the SYNC ring
        # BEHIND xT: ring order is FIFO, so these can't steal the shared
        # ~330GB/s AXI path from xT + the prologue qk slices during warmup
        # (the Tile scheduler reorders instructions, so emission order
        # alone cannot enforce this).
        for c in range(KC):
            nc.sync.dma_start(out=wv_sb[:, c, :], in_=w_v[128 * c:128 * (c + 1), :])
        for p in range(1, KC):  # remaining qk slices, pair order
            for t in (p, KC + p):
                nc.sync.dma_start(
                    out=wqk_sb[:, t, :, :], in_=w_qk[:, TW * t:TW * (t + 1)],
                )
        for c in range(KC):
            nc.sync.dma_start(out=wp_sb[:, c, :], in_=w_p[128 * c:128 * (c + 1), :])
        bap = bias[:, :]
        bias_bcast = bass.AP(
            tensor=bap.tensor, offset=bap.offset,
            ap=[[0, 128]] + list(bap.ap)[1:],
        )
        nc.sync.dma_start(out=bias_sb, in_=bias_bcast)

        v4 = v_sb.rearrange("p t (h e) -> p t h e", e=HD)

        def qkT_ops(t):
            """Closures: 6 accumulation-chunk matmul pairs + the cast copy,
            for interleaving as PE filler inside a pair's mt loop."""
            ps_qk = psA.tile([128, N], F32, tag="ps", name="ps_qk")
            ops = []
            for c in range(KC):
                def chunk(c=c, ps_qk=ps_qk):
                    for s in range(2):
                        nc.tensor.matmul(
                            ps_qk[:, 512 * s:512 * (s + 1)],
                            lhsT=wqk_sb[:, t, c, :],
                            rhs=xT_sb[:, c, 512 * s:512 * (s + 1)],
                            start=(c == 0), stop=(c == KC - 1),
                        )
                ops.append(chunk)

            def fin(ps_qk=ps_qk):
                nc.vector.tensor_copy(out=qk_sb[:, t, :], in_=ps_qk)
            ops.append(fin)
            return ops

        def v_ops(mt):
            ps_v = psA.tile([128, N], F32, tag="ps", name="ps_v")
            ops = []
            for c in range(KC):
                def chunk(c=c, ps_v=ps_v):
                    for lo, sz in ((0, 512), (512, 256)):
                        nc.tensor.matmul(
                            ps_v[:, lo:lo + sz],
                            lhsT=xT_sb[:, c, 128 * mt:128 * (mt + 1)],
                            rhs=wv_sb[:, c, lo:lo + sz],
                            start=(c == 0), stop=(c == KC - 1),
                        )
                ops.append(chunk)

            def fin(ps_v=ps_v):
                nc.vector.tensor_copy(
                    out=v4[:, mt, :, :],
                    in_=ps_v[:, 0:D].rearrange("p (h e) -> p h e", e=HD),
                )
            ops.append(fin)
            return ops

        def proj1_ops(nt):
            """Proj phase 1: contraction chunks 0..PC[nt]-1 + bias, stashed
            bf16. Only legal once pairs 0..PC[nt]-1 are normalized."""
            pc = PC[nt]
            ps_p = psA.tile([128, N], F32, tag="ps", name="ps_p1")
            ops = []
            for c in range(pc):
                def chunk(c=c, ps_p=ps_p):
                    for lo, sz in ((0, 512), (512, 256)):
                        nc.tensor.matmul(
                            ps_p[:, lo:lo + sz],
                            lhsT=attn_sb[:, c, 128 * nt:128 * (nt + 1)],
                            rhs=wp_sb[:, c, lo:lo + sz],
                            start=(c == 0), stop=(c == pc - 1),
                        )
                ops.append(chunk)

            def fin(ps_p=ps_p):
                nc.vector.tensor_add(
                    out=opart_sb[:, nt, :], in0=ps_p[:, 0:D], in1=bias_sb,
                )
            ops.append(fin)
            return ops

        def emit_proj2(nt, store_q):
            pc = PC[nt]
            ps_p = psA.tile([128, N], F32, tag="ps", name="ps_p2")
            for c in range(pc, KC):
                for lo, sz in ((0, 512), (512, 256)):
                    nc.tensor.matmul(
                        ps_p[:, lo:lo + sz],
                        lhsT=attn_sb[:, c, 128 * nt:128 * (nt + 1)],
                        rhs=wp_sb[:, c, lo:lo + sz],
                        start=(c == pc), stop=(c == KC - 1),
                    )
            o_sb = work.tile([128, D], BF16, tag="o_sb", name="o_sb", bufs=3)
            nc.vector.tensor_add(out=o_sb, in0=ps_p[:, 0:D], in1=opart_sb[:, nt, :])
            store_q.dma_start(out=out[128 * nt:128 * (nt + 1), :], in_=o_sb)

        def emit_ST_pair(p, mt):
            """Concurrent K=64 score matmuls for heads 2p (rows 0:64, PE row
            tile 0) and 2p+1 (rows 64:128, row tile 64), then the two exps.
            """
            tq, tk = p, KC + p
            ps_sA = psA.tile([128, N], F32, tag="ps", name="ps_sA")
            ps_sB = psA.tile([128, N], F32, tag="ps", name="ps_sB")
            for s in range(2):
                nc.tensor.matmul(
                    ps_sA[:, 512 * s:512 * (s + 1)],
                    lhsT=qk_sb[0:64, tk, 128 * mt:128 * (mt + 1)],
                    rhs=qk_sb[0:64, tq, 512 * s:512 * (s + 1)],
                    start=True, stop=True,
                )
                nc.tensor.matmul(
                    ps_sB[:, 512 * s:512 * (s + 1)],
                    lhsT=qk_sb[64:128, tk, 128 * mt:128 * (mt + 1)],
                    rhs=qk_sb[64:128, tq, 512 * s:512 * (s + 1)],
                    start=True, stop=True,
                )
            ptA = work.tile([128, N], BF16, tag="pt", name="ptA", bufs=22)
            ptB = work.tile([128, N], BF16, tag="pt", name="ptB", bufs=22)
            nc.scalar.activation(
                out=ptA, in_=ps_sA,
                func=mybir.ActivationFunctionType.Exp, scale=SCALE,
            )
            nc.scalar.activation(
                out=ptB, in_=ps_sB,
                func=mybir.ActivationFunctionType.Exp, scale=SCALE,
            )
            return ptA, ptB

        def emit_PV_pair(p, mt, ptA, ptB, ps_o):
            """Concurrent M=64 PV matmuls: head A -> out partitions 0:64 (PE
            col tile 0), head B -> 64:128 (col tile 64), shared accumulator."""
            hA, hB = 2 * p, 2 * p + 1
            for s in range(2):
                nc.tensor.matmul(
                    ps_o[0:64, 512 * s:512 * (s + 1)],
                    lhsT=v4[:, mt, hA, :],
                    rhs=ptA[:, 512 * s:512 * (s + 1)],
                    start=(mt == 0), stop=(mt == NT - 1),
                )
                nc.tensor.matmul(
                    ps_o[64:128, 512 * s:512 * (s + 1)],
                    lhsT=v4[:, mt, hB, :],
                    rhs=ptB[:, 512 * s:512 * (s + 1)],
                    start=(mt == 0), stop=(mt == NT - 1),
                )

        def emit_sums_window(ps_m, j, pts):
            """4-way concurrent column-strip sums: ones.T @ P for (A,2j)@0,
            (B,2j)@32, (A,2j+1)@64, (B,2j+1)@96, accumulated over windows."""
            ptA0, ptB0 = pts[2 * j]
            ptA1, ptB1 = pts[2 * j + 1]
            quads = ((0, ptA0), (32, ptB0), (64, ptA1), (96, ptB1))
            for s in range(2):
                for strip, pt in quads:
                    nc.tensor.matmul(
                        ps_m[strip:strip + 1, 512 * s:512 * (s + 1)],
                        lhsT=ones_sb[:, 0:1],
                        rhs=pt[:, 512 * s:512 * (s + 1)],
                        start=(j == 0), stop=(j == 3),
                        tile_position=(0, strip),
                    )

        def emit_pair_tail(p, ps_o, pts, cover=()):
            """Pair tail: 4 sums windows sandwiched with the 4 trailing PVs,
            then normalization (selector matmul on the strip copy, 2-row
            reciprocal, broadcasts, multiplies). `cover` closure-lists are
            independent PE work interleaved to hide the path latency."""
            tq = p
            ci = iter(cover)
            ps_m = psA.tile([128, N], F32, tag="ps", name="ps_m")
            emit_sums_window(ps_m, 0, pts)
            emit_PV_pair(p, NT - 4, pts[NT - 4][0], pts[NT - 4][1], ps_o)
            emit_sums_window(ps_m, 1, pts)
            emit_PV_pair(p, NT - 3, pts[NT - 3][0], pts[NT - 3][1], ps_o)
            emit_sums_window(ps_m, 2, pts)
            emit_PV_pair(p, NT - 2, pts[NT - 2][0], pts[NT - 2][1], ps_o)
            emit_PV_pair(p, NT - 1, pts[NT - 1][0], pts[NT - 1][1], ps_o)
            emit_sums_window(ps_m, 3, pts)
            scp = work.tile([128, N], BF16, tag="scp", name="scp")
            nc.vector.tensor_copy(out=scp, in_=ps_m)
            for op in next(ci, []):
                op()
            for s in range(2):
                nc.tensor.matmul(
                    ps_m[0:2, 512 * s:512 * (s + 1)],
                    lhsT=selw_sb[:, 0:2],
                    rhs=scp[:, 512 * s:512 * (s + 1)],
                    start=True, stop=True,
                )
            for op in next(ci, []):
                op()
            rec2 = work.tile([2, N], F32, tag="rec", name="rec2", bufs=2)
            nc.vector.reciprocal_approx_fast(out=rec2, in_=ps_m[0:2, :])
            rbA = work.tile([128, N], F32, tag="rb", name="rbA")
            rbB = work.tile([128, N], F32, tag="rb", name="rbB")
            nc.gpsimd.partition_broadcast(rbA[0:64, :], rec2[0:1, :])
            # partition_broadcast reads base partition 0 only; hop head B's
            # reciprocal row down from partition 1 via DMA first.
            recB = work.tile([1, N], F32, tag="recB", name="recB", bufs=2)
            nc.sync.dma_start(out=recB, in_=rec2[1:2, :])
            nc.gpsimd.partition_broadcast(rbB, recB)
            nc.vector.tensor_mul(
                out=attn_sb[0:64, tq, :], in0=ps_o[0:64, :], in1=rbA[0:64, :],
            )
            nc.vector.tensor_mul(
                out=attn_sb[64:128, tq, :], in0=ps_o[64:128, :], in1=rbB[64:128, :],
            )
            for t in ci:
                for op in t:
                    op()

        # ---- schedule ----
        # PE warm-up: the HAM p-state ladder needs several us of continuous
        # matmul activity to reach full clock, and the PE otherwise idles
        # from engine-init (~7us) until the first inputs land (~12us).
        # Burn that window on dummy matmuls so the prologue runs warm.
        warm_sb = const.tile([128, 512], BF16)
        nc.gpsimd.memset(warm_sb, 1.0)
        ps_w = psA.tile([128, N], F32, tag="ps", name="ps_warm")
        for _ in range(10):
            nc.tensor.matmul(
                ps_w[:, 0:512], lhsT=warm_sb[:, 0:128], rhs=warm_sb[:, 0:512],
                start=True, stop=True,
            )
        # Prologue: q/k chains for pair 0, chunk-interleaved to match the
        # DMA arrival order of xT chunks.
        ops0, ops6 = qkT_ops(0), qkT_ops(KC)
        for c in range(KC):
            ops0[c]()
            ops6[c]()
        ops0[KC]()
        ops6[KC]()

        # PE filler per pair. All v chains must complete within pair 0
        # (pair-0 PVs consume them at mt pace); qkT chains for pair p+1
        # complete within pair p; proj phase 1 per PC[] above (norm(q) for
        # pair q is emitted in q's tail, before the next pair's filler).
        fillers = {pp: [] for pp in range(NP)}
        for mt in range(NT):
            fillers[0] += v_ops(mt)
        fillers[0] += qkT_ops(1) + qkT_ops(KC + 1)
        for pp in range(1, NP - 1):
            fillers[pp] = qkT_ops(pp + 1) + qkT_ops(KC + pp + 1)
        fillers[4] += proj1_ops(0) + proj1_ops(1) + proj1_ops(2)
        fillers[5] = proj1_ops(3) + proj1_ops(4) + proj1_ops(5)

        store_qs = [nc.sync, nc.scalar, nc.gpsimd]
        bridge = []   # pts of the next pair's first mts, pre-emitted
        for p in range(NP):
            ps_o = psB.tile([128, N], F32, tag="pso", name="ps_o")
            fl = fillers[p]
            fi = 0
            pts = bridge
            lo = 1 if p == 0 else 3
            for mt in range(len(pts), NT):
                ptA, ptB = emit_ST_pair(p, mt)
                pts.append((ptA, ptB))
                if mt >= LAG:
                    k = mt - LAG
                    emit_PV_pair(p, k, pts[k][0], pts[k][1], ps_o)
                if mt >= lo:
                    if p == 0:
                        # pair 0 is overloaded (all v chains): keep the ops
                        # spread so v[mt] lands just in time for its PV
                        want = min(len(fl), ((mt - lo + 1) * len(fl)
                                   + NT - lo - 1) // (NT - lo))
                    else:
                        # burst: short PSUM slot holds; the scheduler
                        # interleaves execution by dependencies anyway
                        want = len(fl)
                    while fi < want:
                        fl[fi]()
                        fi += 1
            while fi < len(fl):
                fl[fi]()
                fi += 1
            # bridge the exp stream: next pair's first two ST pairs run
            # while this pair's tail occupies the PE.
            if p < NP - 1:
                bridge = [emit_ST_pair(p + 1, 0), emit_ST_pair(p + 1, 1),
                          emit_ST_pair(p + 1, 2)]
                emit_pair_tail(p, ps_o, pts)
            else:
                emit_pair_tail(
                    p, ps_o, pts,
                    cover=(
                        proj1_ops(6), proj1_ops(7),
                        [lambda nt=nt: emit_proj2(nt, store_qs[nt % 3])
                         for nt in range(3)],
                    ),
                )

        for nt in range(3, NT):
            emit_proj2(nt, store_qs[nt % 3])

    nc.compile()
    return nc


def _get_nc():
    if "nc" not in _CACHE:
        _CACHE["nc"] = _build_nc()
    return _CACHE["nc"]


def _make_in_maps(x, W_qkv, W_proj, b_proj):
    bf = ml_dtypes.bfloat16
    x = np.asarray(x, dtype=np.float32)
    W_qkv = np.asarray(W_qkv, dtype=np.float32)
    W_proj = np.asarray(W_proj, dtype=np.float32)
    b_proj = np.asarray(b_proj, dtype=np.float32)
    # stage w_qk to [p, t, c, w]: each t-slice is one contiguous-row DMA
    w_qk = np.ascontiguousarray(
        W_qkv[:, :2 * D].reshape(KC, 128, 2 * KC, 128)
        .transpose(1, 2, 0, 3).reshape(128, 2 * KC * KC * 128)
    ).astype(bf)
    w_v = np.ascontiguousarray(W_qkv[:, 2 * D:]).astype(bf)
    w_p = W_proj.astype(bf)
    bias = b_proj.reshape(1, D)
    return [
        {
            "xT": np.ascontiguousarray(x[b].T).astype(bf),
            "w_qk": w_qk,
            "w_v": w_v,
            "w_p": w_p,
            "bias": bias,
        }
        for b in range(NCORES)
    ]


def run(x, W_qkv, W_proj, b_proj, trace=False):
    nc = _get_nc()
    in_maps = _make_in_maps(x, W_qkv, W_proj, b_proj)
    res = run_bass_kernel_spmd(nc, in_maps, core_ids=list(range(NCORES)), trace=trace)
    out = np.stack([res.results[b]["out"] for b in range(NCORES)], axis=0)
    return out.astype(np.float32), res


def kernel(x, W_qkv, W_proj, b_proj):
    out, _ = run(x, W_qkv, W_proj, b_proj, trace=False)
    return out


# revision 51
# speedup vs baseline: 1.2217x; 1.0293x over previous
"""Trainium2 Bass kernel: 12-head self-attention (B=8, N=1024, D=768).

Sharding: data-parallel over batch - one batch element per NeuronCore,
weights replicated on all 8 cores, no collectives.

Per-core dataflow (matmuls bf16 operands, fp32 PSUM accumulation).
Heads are processed in PAIRS (2p, 2p+1): head 2p lives on SBUF
partitions 0..63, head 2p+1 on 64..127 of the same qk chunk, so the
K=64 score matmuls of the two heads land in different PE row-groups
(row tiling) and execute CONCURRENTLY; the M=64 PV matmuls of the two
heads land in different PE column-groups (col tiling, out partitions
0..63 vs 64..127 of a shared accumulator) and also run concurrently.
Softmax denominators come from M=1 ones-matmuls batched 4 per window
into distinct 32-wide PSUM column strips (4-way concurrent), PSUM-
accumulated over the pair, combined by one M=2 selector matmul + a
2-row reciprocal, broadcast (gpsimd for head A, a DMA partition hop +
gpsimd for head B), and multiplied into attn_sb.

Scheduling: software pipeline paced by the ACT (exp) stream. PV lags
ST/exp by 4 mt steps. At each pair boundary the NEXT pair's first two
ST pairs are emitted ("bridge") before this pair's tail (4 sums
windows + 4 trailing PVs + normalization), so the exp stream never
starves while the tail occupies the PE. qkT and v chains fill the
remaining PE slack; the output projection is split, with chunks
0..PC-1 run as filler once the needed pairs are normalized (partials
stashed bf16 with bias) and the rest forming a short tail with bf16
stores spread over three DMA queues.

PSUM (8 banks): psA = 3 rotating [128,1024] slots (ST_A/ST_B of the
current mt + one filler-chain/sums slot) = 6 banks; psB = the pair's
PV accumulator = 2 banks.

w_qk is HOST-STAGED to [p, t, c, w] layout so each 128-column t-slice
(the weights one head pair needs) is one contiguous-row DMA; the two
prologue slices go on the scalar queue (cheap issue, the ACTIVATE
stream behind them is not delayed), the rest stream on gpsimd behind
wv in pair order.
"""

from contextlib import ExitStack

import numpy as np
import ml_dtypes

import concourse.bacc as bacc
import concourse.bass as bass
import concourse.mybir as mybir
import concourse.tile as tile
from concourse.bass_utils import run_bass_kernel_spmd

B, N, D = 8, 1024, 768
H, HD = 12, 64
NP = H // 2            # 6 head pairs
SCALE = HD ** -0.5
KC = D // 128          # 6 contraction chunks of 128
NT = N // 128          # 8 token tiles of 128
F32 = mybir.dt.float32
BF16 = mybir.dt.bfloat16
NCORES = 8
LAG = 4                # PV trails ST/exp by this many mt steps
# proj phase-1 chunk count per token tile: nt 0-2 run chunks 0-3 as
# pair-4 filler (needs norm(3)), nt 3-7 run chunks 0-4 as pair-5
# filler (needs norm(4)); the rest is the tail.
PC = [4, 4, 4, 5, 5, 5, 5, 5]

_CACHE = {}


def _build_nc():
    nc = bacc.Bacc(None, target_bir_lowering=False)
    xT = nc.dram_tensor("xT", [D, N], BF16, kind="ExternalInput")
    # host-staged: [p, t, c, w] -> W_qk[128c+p, 128t+w], flattened [128, 9216]
    w_qk = nc.dram_tensor("w_qk", [128, 2 * KC * KC * 128], BF16, kind="ExternalInput")
    w_v = nc.dram_tensor("w_v", [D, D], BF16, kind="ExternalInput")
    w_p = nc.dram_tensor("w_p", [D, D], BF16, kind="ExternalInput")
    bias = nc.dram_tensor("bias", [1, D], F32, kind="ExternalInput")
    out = nc.dram_tensor("out", [N, D], BF16, kind="ExternalOutput")

    with ExitStack() as ctx:
        tc = ctx.enter_context(tile.TileContext(nc))
        const = ctx.enter_context(tc.tile_pool(name="const", bufs=1))
        work = ctx.enter_context(tc.tile_pool(name="work", bufs=2))
        psA = ctx.enter_context(tc.tile_pool(name="psA", bufs=3, space="PSUM"))
        psB = ctx.enter_context(tc.tile_pool(name="psB", bufs=1, space="PSUM"))

        xT_sb = const.tile([128, KC, N], BF16)
        wqk_sb = const.tile([128, 2 * KC, KC, 128], BF16)   # [p, t, c, w]
        wv_sb = const.tile([128, KC, D], BF16)
        wp_sb = const.tile([128, KC, D], BF16)
        bias_sb = const.tile([128, D], F32)
        qk_sb = const.tile([128, 2 * KC, N], BF16)   # chunks 0-5: qT, 6-11: kT
        v_sb = const.tile([128, NT, D], BF16)        # per-mt v, head-major
        attn_sb = const.tile([128, KC, N], BF16)     # attn_out^T, normalized
        opart_sb = const.tile([128, NT, D], BF16)    # proj partials + bias
        ones_sb = const.tile([128, 1], BF16)
        selw_sb = const.tile([128, 2], BF16)

        # --- input DMAs ---
        TW = KC * 128  # elements per t-slice per partition row
        for c in range(KC):
            nc.sync.dma_start(out=xT_sb[:, c, :], in_=xT[128 * c:128 * (c + 1), :])
        for t in (0, KC):  # prologue q/k slices for pair 0
            nc.scalar.dma_start(
                out=wqk_sb[:, t, :, :], in_=w_qk[:, TW * t:TW * (t + 1)],
            )
        nc.gpsimd.memset(ones_sb, 1.0)
        # selector weights: col 0 picks rows {0,64} (head A strip partials),
        # col 1 picks rows {32,96} (head B).
        nc.gpsimd.memset(selw_sb, 0.0)
        nc.gpsimd.memset(selw_sb[0:1, 0:1], 1.0)
        nc.gpsimd.memset(selw_sb[64:65, 0:1], 1.0)
        nc.gpsimd.memset(selw_sb[32:33, 1:2], 1.0)
        nc.gpsimd.memset(selw_sb[96:97, 1:2], 1.0)
        # Everything not needed in the first ~25us rides the SYNC ring
        # BEHIND xT: ring order is FIFO, so these can't steal the shared
        # ~330GB/s AXI path from xT + the prologue qk slices during warmup
        # (the Tile scheduler reorders instructions, so emission order
        # alone cannot enforce this).
        for c in range(KC):
            nc.sync.dma_start(out=wv_sb[:, c, :], in_=w_v[128 * c:128 * (c + 1), :])
        for p in range(1, KC):  # remaining qk slices, pair order
            for t in (p, KC + p):
                nc.sync.dma_start(
                    out=wqk_sb[:, t, :, :], in_=w_qk[:, TW * t:TW * (t + 1)],
                )
        for c in range(KC):
            nc.sync.dma_start(out=wp_sb[:, c, :], in_=w_p[128 * c:128 * (c + 1), :])
        bap = bias[:, :]
        bias_bcast = bass.AP(
            tensor=bap.tensor, offset=bap.offset,
            ap=[[0, 128]] + list(bap.ap)[1:],
        )
        nc.sync.dma_start(out=bias_sb, in_=bias_bcast)

        v4 = v_sb.rearrange("p t (h e) -> p t h e", e=HD)

        def qkT_ops(t):
            """Closures: 6 accumulation-chunk matmul pairs + the cast copy,
            for interleaving as PE filler inside a pair's mt loop."""
            ps_qk = psA.tile([128, N], F32, tag="ps", name="ps_qk")
            ops = []
            for c in range(KC):
                def chunk(c=c, ps_qk=ps_qk):
                    for s in range(2):
                        nc.tensor.matmul(
                            ps_qk[:, 512 * s:512 * (s + 1)],
                            lhsT=wqk_sb[:, t, c, :],
                            rhs=xT_sb[:, c, 512 * s:512 * (s + 1)],
                            start=(c == 0), stop=(c == KC - 1),
                        )
                ops.append(chunk)

            def fin(ps_qk=ps_qk):
                nc.vector.tensor_copy(out=qk_sb[:, t, :], in_=ps_qk)
            ops.append(fin)
            return ops

        def v_ops(mt):
            ps_v = psA.tile([128, N], F32, tag="ps", name="ps_v")
            ops = []
            for c in range(KC):
                def chunk(c=c, ps_v=ps_v):
                    for lo, sz in ((0, 512), (512, 256)):
                        nc.tensor.matmul(
                            ps_v[:, lo:lo + sz],
                            lhsT=xT_sb[:, c, 128 * mt:128 * (mt + 1)],
                            rhs=wv_sb[:, c, lo:lo + sz],
                            start=(c == 0), stop=(c == KC - 1),
                        )
                ops.append(chunk)

            def fin(ps_v=ps_v):
                nc.vector.tensor_copy(
                    out=v4[:, mt, :, :],
                    in_=ps_v[:, 0:D].rearrange("p (h e) -> p h e", e=HD),
                )
            ops.append(fin)
            return ops

        def proj1_ops(nt):
            """Proj phase 1: contraction chunks 0..PC[nt]-1 + bias, stashed
            bf16. Only legal once pairs 0..PC[nt]-1 are normalized."""
            pc = PC[nt]
            ps_p = psA.tile([128, N], F32, tag="ps", name="ps_p1")
            ops = []
            for c in range(pc):
                def chunk(c=c, ps_p=ps_p):
                    for lo, sz in ((0, 512), (512, 256)):
                        nc.tensor.matmul(
                            ps_p[:, lo:lo + sz],
                            lhsT=attn_sb[:, c, 128 * nt:128 * (nt + 1)],
                            rhs=wp_sb[:, c, lo:lo + sz],
                            start=(c == 0), stop=(c == pc - 1),
                        )
                ops.append(chunk)

            def fin(ps_p=ps_p):
                nc.vector.tensor_add(
                    out=opart_sb[:, nt, :], in0=ps_p[:, 0:D], in1=bias_sb,
                )
            ops.append(fin)
            return ops

        def emit_proj2(nt, store_q):
            pc = PC[nt]
            ps_p = psA.tile([128, N], F32, tag="ps", name="ps_p2")
            for c in range(pc, KC):
                for lo, sz in ((0, 512), (512, 256)):
                    nc.tensor.matmul(
                        ps_p[:, lo:lo + sz],
                        lhsT=attn_sb[:, c, 128 * nt:128 * (nt + 1)],
                        rhs=wp_sb[:, c, lo:lo + sz],
                        start=(c == pc), stop=(c == KC - 1),
                    )
            o_sb = work.tile([128, D], BF16, tag="o_sb", name="o_sb", bufs=3)
            nc.vector.tensor_add(out=o_sb, in0=ps_p[:, 0:D], in1=opart_sb[:, nt, :])
            store_q.dma_start(out=out[128 * nt:128 * (nt + 1), :], in_=o_sb)

        def emit_ST_pair(p, mt):
            """Concurrent K=64 score matmuls for heads 2p (rows 0:64, PE row
            tile 0) and 2p+1 (rows 64:128, row tile 64), then the two exps.
            """
            tq, tk = p, KC + p
            ps_sA = psA.tile([128, N], F32, tag="ps", name="ps_sA")
            ps_sB = psA.tile([128, N], F32, tag="ps", name="ps_sB")
            for s in range(2):
                nc.tensor.matmul(
                    ps_sA[:, 512 * s:512 * (s + 1)],
                    lhsT=qk_sb[0:64, tk, 128 * mt:128 * (mt + 1)],
                    rhs=qk_sb[0:64, tq, 512 * s:512 * (s + 1)],
                    start=True, stop=True,
                )
                nc.tensor.matmul(
                    ps_sB[:, 512 * s:512 * (s + 1)],
                    lhsT=qk_sb[64:128, tk, 128 * mt:128 * (mt + 1)],
                    rhs=qk_sb[64:128, tq, 512 * s:512 * (s + 1)],
                    start=True, stop=True,
                )
            ptA = work.tile([128, N], BF16, tag="pt", name="ptA", bufs=22)
            ptB = work.tile([128, N], BF16, tag="pt", name="ptB", bufs=22)
            nc.scalar.activation(
                out=ptA, in_=ps_sA,
                func=mybir.ActivationFunctionType.Exp, scale=SCALE,
            )
            nc.scalar.activation(
                out=ptB, in_=ps_sB,
                func=mybir.ActivationFunctionType.Exp, scale=SCALE,
            )
            return ptA, ptB

        def emit_PV_pair(p, mt, ptA, ptB, ps_o):
            """Concurrent M=64 PV matmuls: head A -> out partitions 0:64 (PE
            col tile 0), head B -> 64:128 (col tile 64), shared accumulator."""
            hA, hB = 2 * p, 2 * p + 1
            for s in range(2):
                nc.tensor.matmul(
                    ps_o[0:64, 512 * s:512 * (s + 1)],
                    lhsT=v4[:, mt, hA, :],
                    rhs=ptA[:, 512 * s:512 * (s + 1)],
                    start=(mt == 0), stop=(mt == NT - 1),
                )
                nc.tensor.matmul(
                    ps_o[64:128, 512 * s:512 * (s + 1)],
                    lhsT=v4[:, mt, hB, :],
                    rhs=ptB[:, 512 * s:512 * (s + 1)],
                    start=(mt == 0), stop=(mt == NT - 1),
                )

        def emit_sums_window(ps_m, j, pts):
            """4-way concurrent column-strip sums: ones.T @ P for (A,2j)@0,
            (B,2j)@32, (A,2j+1)@64, (B,2j+1)@96, accumulated over windows."""
            ptA0, ptB0 = pts[2 * j]
            ptA1, ptB1 = pts[2 * j + 1]
            quads = ((0, ptA0), (32, ptB0), (64, ptA1), (96, ptB1))
            for s in range(2):
                for strip, pt in quads:
                    nc.tensor.matmul(
                        ps_m[strip:strip + 1, 512 * s:512 * (s + 1)],
                        lhsT=ones_sb[:, 0:1],
                        rhs=pt[:, 512 * s:512 * (s + 1)],
                        start=(j == 0), stop=(j == 3),
                        tile_position=(0, strip),
                    )

        def emit_pair_tail(p, ps_o, pts, cover=()):
            """Pair tail: 4 sums windows sandwiched with the 4 trailing PVs,
            then normalization (selector matmul on the strip copy, 2-row
            reciprocal, broadcasts, multiplies). `cover` closure-lists are
            independent PE work interleaved to hide the path latency."""
            tq = p
            ci = iter(cover)
            ps_m = psA.tile([128, N], F32, tag="ps", name="ps_m")
            emit_sums_window(ps_m, 0, pts)
            emit_PV_pair(p, NT - 4, pts[NT - 4][0], pts[NT - 4][1], ps_o)
            emit_sums_window(ps_m, 1, pts)
            emit_PV_pair(p, NT - 3, pts[NT - 3][0], pts[NT - 3][1], ps_o)
            emit_sums_window(ps_m, 2, pts)
            emit_PV_pair(p, NT - 2, pts[NT - 2][0], pts[NT - 2][1], ps_o)
            emit_PV_pair(p, NT - 1, pts[NT - 1][0], pts[NT - 1][1], ps_o)
            emit_sums_window(ps_m, 3, pts)
            scp = work.tile([128, N], BF16, tag="scp", name="scp")
            nc.vector.tensor_copy(out=scp, in_=ps_m)
            for op in next(ci, []):
                op()
            for s in range(2):
                nc.tensor.matmul(
                    ps_m[0:2, 512 * s:512 * (s + 1)],
                    lhsT=selw_sb[:, 0:2],
                    rhs=scp[:, 512 * s:512 * (s + 1)],
                    start=True, stop=True,
                )
            for op in next(ci, []):
                op()
            rec2 = work.tile([2, N], F32, tag="rec", name="rec2", bufs=2)
            nc.vector.reciprocal_approx_fast(out=rec2, in_=ps_m[0:2, :])
            rbA = work.tile([128, N], F32, tag="rb", name="rbA")
            rbB = work.tile([128, N], F32, tag="rb", name="rbB")
            nc.gpsimd.partition_broadcast(rbA[0:64, :], rec2[0:1, :])
            # partition_broadcast reads base partition 0 only; hop head B's
            # reciprocal row down from partition 1 via DMA first.
            recB = work.tile([1, N], F32, tag="recB", name="recB", bufs=2)
            nc.sync.dma_start(out=recB, in_=rec2[1:2, :])
            nc.gpsimd.partition_broadcast(rbB, recB)
            nc.vector.tensor_mul(
                out=attn_sb[0:64, tq, :], in0=ps_o[0:64, :], in1=rbA[0:64, :],
            )
            nc.vector.tensor_mul(
                out=attn_sb[64:128, tq, :], in0=ps_o[64:128, :], in1=rbB[64:128, :],
            )
            for t in ci:
                for op in t:
                    op()

        # ---- schedule ----
        # PE warm-up: the HAM p-state ladder needs several us of continuous
        # matmul activity to reach full clock, and the PE otherwise idles
        # from engine-init (~7us) until the first inputs land (~12us).
        # Burn that window on dummy matmuls so the prologue runs warm.
        warm_sb = const.tile([128, 512], BF16)
        nc.gpsimd.memset(warm_sb, 1.0)
        ps_w = psA.tile([128, N], F32, tag="ps", name="ps_warm")
        for _ in range(10):
            nc.tensor.matmul(
                ps_w[:, 0:512], lhsT=warm_sb[:, 0:128], rhs=warm_sb[:, 0:512],
                start=True, stop=True,
            )
        # Prologue: q/k chains for pair 0, chunk-interleaved to match the
        # DMA arrival order of xT chunks.
        ops0, ops6 = qkT_ops(0), qkT_ops(KC)
        for c in range(KC):
            ops0[c]()
            ops6[c]()
        ops0[KC]()
        ops6[KC]()

        # PE filler per pair. All v chains must complete within pair 0
        # (pair-0 PVs consume them at mt pace); qkT chains for pair p+1
        # complete within pair p; proj phase 1 per PC[] above (norm(q) for
        # pair q is emitted in q's tail, before the next pair's filler).
        fillers = {pp: [] for pp in range(NP)}
        for mt in range(NT):
            fillers[0] += v_ops(mt)
        fillers[0] += qkT_ops(1) + qkT_ops(KC + 1)
        for pp in range(1, NP - 1):
            fillers[pp] = qkT_ops(pp + 1) + qkT_ops(KC + pp + 1)
        fillers[4] += proj1_ops(0) + proj1_ops(1) + proj1_ops(2)
        fillers[5] = proj1_ops(3) + proj1_ops(4) + proj1_ops(5)

        store_qs = [nc.sync, nc.scalar, nc.gpsimd]
        bridge = []   # pts of the next pair's first mts, pre-emitted
        for p in range(NP):
            ps_o = psB.tile([128, N], F32, tag="pso", name="ps_o")
            fl = fillers[p]
            fi = 0
            pts = bridge
            lo = 1 if p == 0 else 3
            for mt in range(len(pts), NT):
                ptA, ptB = emit_ST_pair(p, mt)
                pts.append((ptA, ptB))
                if mt >= LAG:
                    k = mt - LAG
                    emit_PV_pair(p, k, pts[k][0], pts[k][1], ps_o)
                if mt >= lo:
                    want = min(len(fl), ((mt - lo + 1) * len(fl) + NT - lo - 1)
                               // (NT - lo))
                    while fi < want:
                        fl[fi]()
                        fi += 1
            while fi < len(fl):
                fl[fi]()
                fi += 1
            # bridge the exp stream: next pair's first two ST pairs run
            # while this pair's tail occupies the PE.
            if p < NP - 1:
                bridge = [emit_ST_pair(p + 1, 0), emit_ST_pair(p + 1, 1),
                          emit_ST_pair(p + 1, 2)]
                emit_pair_tail(p, ps_o, pts)
            else:
                emit_pair_tail(
                    p, ps_o, pts,
                    cover=(
                        proj1_ops(6), proj1_ops(7),
                        [lambda nt=nt: emit_proj2(nt, store_qs[nt % 3])
                         for nt in range(3)],
                    ),
                )

        for nt in range(3, NT):
            emit_proj2(nt, store_qs[nt % 3])

    nc.compile()
    return nc


def _get_nc():
    if "nc" not in _CACHE:
        _CACHE["nc"] = _build_nc()
    return _CACHE["nc"]


def _make_in_maps(x, W_qkv, W_proj, b_proj):
    bf = ml_dtypes.bfloat16
    x = np.asarray(x, dtype=np.float32)
    W_qkv = np.asarray(W_qkv, dtype=np.float32)
    W_proj = np.asarray(W_proj, dtype=np.float32)
    b_proj = np.asarray(b_proj, dtype=np.float32)
    # stage w_qk to [p, t, c, w]: each t-slice is one contiguous-row DMA
    w_qk = np.ascontiguousarray(
        W_qkv[:, :2 * D].reshape(KC, 128, 2 * KC, 128)
        .transpose(1, 2, 0, 3).reshape(128, 2 * KC * KC * 128)
    ).astype(bf)
    w_v = np.ascontiguousarray(W_qkv[:, 2 * D:]).astype(bf)
    w_p = W_proj.astype(bf)
    bias = b_proj.reshape(1, D)
    return [
        {
            "xT": np.ascontiguousarray(x[b].T).astype(bf),
            "w_qk": w_qk,
            "w_v": w_v,
            "w_p": w_p,
            "bias": bias,
        }
        for b in range(NCORES)
    ]


def run(x, W_qkv, W_proj, b_proj, trace=False):
    nc = _get_nc()
    in_maps = _make_in_maps(x, W_qkv, W_proj, b_proj)
    res = run_bass_kernel_spmd(nc, in_maps, core_ids=list(range(NCORES)), trace=trace)
    out = np.stack([res.results[b]["out"] for b in range(NCORES)], axis=0)
    return out.astype(np.float32), res


def kernel(x, W_qkv, W_proj, b_proj):
    out, _ = run(x, W_qkv, W_proj, b_proj, trace=False)
    return out
